# revision 25
# baseline (speedup 1.0000x reference)
"""Causal multi-head self-attention on 8 trn2 NeuronCores.

Sharding: 8 cores = 4 batch x 2 head-groups. Core i handles batch i//2 and
heads (i%2)*8 .. (i%2)*8+8 (8 of 16 heads, 512 of 1024 d_model columns).
Each core computes a full (2048, 1024) partial output (its head group pushed
through its w_proj row-slice); the host sums the two partials per batch
element (the tensor-parallel all-reduce done host-side).

All inputs are converted to bf16 on the host; x is host-transposed so x^T
DMAs straight into SBUF (no PE transposes, no staging casts). Per-core
dataflow, everything in transposed layout:
  Q^T, K^T   : w_q/w_k stationary, x^T moving  -> [cols, seq] bf16
  V_aug      : x^T stationary, w_v moving      -> natural [seq, cols] bf16
               + a ones-column per head so the softmax denominator rides the
               AV matmul as output row 64
  S^T        : K^T stationary, Q^T moving; the two heads of a pair go to PE
               row groups 0-63 / 64-127 (K=64) and run concurrently; two
               k-tiles of S land in one 2-bank PSUM tile [128, 1024]
  P^T        : one exp per (head, kt-pair) over the 2-bank tile on ScalarE
               (no max-subtraction: |S|*rsqrt < ~10); causal = skip k>q
               chunks, zero invalid prefixes, 0/1 mask mul on diag blocks
  O^T_aug    : V_aug stationary, P^T moving, accumulated over k-tiles in PSUM
  normalize  : denominators for (h0, h1) of a q-chunk drain to two SBUF
               partitions; 1/d via one reciprocal_approx_fast (DVE), then a
               single K=2 matmul against a 0/1 selector broadcasts both
               reciprocal rows over the pair's 128 partitions; one DVE mul
  out        : O^T stationary, w_proj rows moving -> natural [seq, 1024] f32

Schedule: one strictly-ordered PE chain (dep edges pin every matmul). Per
kt-pair step: 4 S matmuls (pairs packed via row groups), filler units
(next pair's QKV chunks, V tail, normalizes, and for the last head pair the
projection s-tiles), then the previous step's 4 AV matmuls. ScalarE chews
exp one step behind the S matmuls; AV trails exp by a step.
"""

import numpy as np
import ml_dtypes

import concourse.bass as bass
import concourse.mybir as mybir
import concourse.tile as tile
from concourse import bacc
from concourse.bass_utils import run_bass_kernel_spmd
from concourse.masks import make_upper_triangular
from concourse.tile_rust import add_dep_helper
from collections import deque

F32 = mybir.dt.float32
BF16 = mybir.dt.bfloat16
AF = mybir.ActivationFunctionType

SEQ = 2048
DM = 1024
COLS = 512          # head-cols per core (8 heads x 64)
HD = 64
P = 128
N_CORES = 8
RSQRT = 0.125       # 1/sqrt(64)

SEQ_T = SEQ // P    # 16 seq tiles
DM_T = DM // P      # 8 d_model tiles
QC = 512            # q-chunk (PSUM free size)
N_QC = SEQ // QC    # 4 q chunks
NP = 4              # head pairs per core


def _build_core_program():
    nc = bacc.Bacc(
        "TRN2", target_bir_lowering=False, debug=False, num_devices=N_CORES
    )
    xT = nc.dram_tensor("xT", [DM, SEQ], BF16, kind="ExternalInput").ap()
    sel = nc.dram_tensor("sel", [HD, P], F32, kind="ExternalInput").ap()
    wq = nc.dram_tensor("wq", [DM, COLS], BF16, kind="ExternalInput").ap()
    wk = nc.dram_tensor("wk", [DM, COLS], BF16, kind="ExternalInput").ap()
    wv = nc.dram_tensor("wv", [DM, COLS], BF16, kind="ExternalInput").ap()
    wp = nc.dram_tensor("wp", [COLS, DM], BF16, kind="ExternalInput").ap()
    out = nc.dram_tensor("out", [SEQ, DM], F32, kind="ExternalOutput").ap()

    with tile.TileContext(nc) as tc:
        _emit(tc, xT, sel, wq, wk, wv, wp, out)
    nc.compile()
    return nc


def _emit(tc, xT, sel, wq, wk, wv, wp, out):
    nc = tc.nc

    # strict PE order: every matmul chains onto the previous one (order-only
    # edge, no semaphore) so the scheduler cannot interpose PE work between
    # an S row-group pair, which would break their concurrent execution
    chain = [None]

    def pin(mm):
        if chain[0] is not None:
            add_dep_helper(mm.ins, chain[0], sync=False, reason="pe-chain")
        chain[0] = mm.ins

    # --- pools ------------------------------------------------------------
    const_pool = tc.alloc_tile_pool(name="const", bufs=1)
    ps_s = tc.alloc_tile_pool(name="ps_s", bufs=2, space="PSUM")      # 4 banks
    psum_mm = tc.alloc_tile_pool(name="psum_mm", bufs=2, space="PSUM")  # 2
    psum_acc = tc.alloc_tile_pool(name="psum_acc", bufs=2, space="PSUM")  # 2

    # --- constants --------------------------------------------------------
    mask01 = const_pool.tile([P, P], BF16, tag="mask01")
    # 1.0 where free-idx (q) >= partition-idx (k), else 0 — causal in S^T
    make_upper_triangular(nc, mask01[:], val=1.0, diag=True)
    cstage = const_pool.tile([P, QC], F32, tag="cstage")
    nc.vector.memset(cstage[:], 0.0)
    zeros512 = const_pool.tile([P, QC], BF16, tag="zeros512")
    nc.vector.tensor_copy(zeros512[:], cstage[:])
    nc.vector.memset(cstage[:, 0:8], 1.0)
    ones8 = const_pool.tile([P, 8], BF16, tag="ones8")
    nc.vector.tensor_copy(ones8[:], cstage[:, 0:8])
    # 0/1 selector for the reciprocal broadcast (host-built): contraction row
    # 0 carries h0's reciprocal to out partitions 0-63, row 32 carries h1's
    # to 64-127 (K=64: odd contraction sizes split into two slow matmuls)
    sel64 = const_pool.tile([HD, P], F32, tag="sel64")
    nc.sync.dma_start(sel64[:], sel[:, :])

    # --- persistent SBUF --------------------------------------------------
    xt_pool = tc.alloc_tile_pool(name="xt", bufs=1)
    xt = xt_pool.tile([P, DM_T * SEQ], BF16, tag="xt")  # x^T, d-tile major
    wv_pool = tc.alloc_tile_pool(name="wv", bufs=1)
    wv_sb = wv_pool.tile([P, DM_T * COLS], BF16, tag="wv_sb")
    vaug_pool = tc.alloc_tile_pool(name="vaug", bufs=1)
    vaug = vaug_pool.tile([P, SEQ_T * 8 * (HD + 1)], BF16, tag="vaug")
    vaug_v = vaug[:].rearrange("p (s h e) -> p s h e", s=SEQ_T, h=8)
    oT_pool = tc.alloc_tile_pool(name="oT", bufs=1)
    oT = oT_pool.tile([P, NP * SEQ], BF16, tag="oT")
    wp_pool = tc.alloc_tile_pool(name="wp", bufs=1)
    wp_sb = wp_pool.tile([P, 4 * DM], BF16, tag="wp_sb")

    qk_pool = tc.alloc_tile_pool(name="qk", bufs=2)
    wqk_pool = tc.alloc_tile_pool(name="wqk", bufs=2)
    pt_pool = tc.alloc_tile_pool(name="pt", bufs=6)
    dcol_pool = tc.alloc_tile_pool(name="dcol", bufs=2)
    rcol_pool = tc.alloc_tile_pool(name="rcol", bufs=2)
    ostage_pool = tc.alloc_tile_pool(name="ostage", bufs=3)

    # --- input DMAs (bf16, direct into final layout) ----------------------
    for d in range(DM_T):
        nc.sync.dma_start(
            xt[:, d * SEQ : (d + 1) * SEQ], xT[d * P : (d + 1) * P, :]
        )
        nc.sync.dma_start(
            wv_sb[:, d * COLS : (d + 1) * COLS], wv[d * P : (d + 1) * P, :]
        )
    for c in range(4):
        nc.sync.dma_start(
            wp_sb[:, c * DM : (c + 1) * DM], wp[c * P : (c + 1) * P, :]
        )

    # ===== V (natural layout) + ones columns ==============================
    def v_mms(s):
        ps = psum_mm.tile([P, QC], F32, tag="mm", name=f"vps_{s}")
        for d in range(DM_T):
            mm = nc.tensor.matmul(
                ps[:],
                xt[:, d * SEQ + s * P : d * SEQ + (s + 1) * P],
                wv_sb[:, d * COLS : (d + 1) * COLS],
                start=(d == 0),
                stop=(d == DM_T - 1),
            )
            pin(mm)
            if d % 2 == 1 and d < DM_T - 1:
                yield
        # drain on ScalarE (idle outside attention, slack within) to keep
        # the DVE queue short
        nc.scalar.copy(
            vaug_v[:, s, :, 0:HD], ps[:].rearrange("p (h e) -> p h e", h=8)
        )
        nc.vector.tensor_copy(
            vaug_v[:, s, :, HD : HD + 1],
            ones8[:].rearrange("p (a b) -> p a b", b=1),
        )
        yield

    for s in range(12):  # first 12 seq-tiles upfront; rest as fillers
        for _ in v_mms(s):
            pass

    # ===== Q^T / K^T pair machinery =======================================
    def qk_pair_dma(hp):
        wq_sb = wqk_pool.tile([P, DM_T * P], BF16, tag="wq_sb", name=f"wq{hp}")
        wk_sb = wqk_pool.tile([P, DM_T * P], BF16, tag="wk_sb", name=f"wk{hp}")
        nc.sync.dma_start(
            wq_sb[:].rearrange("p (d c) -> p d c", d=DM_T),
            wq[:, hp * P : (hp + 1) * P].rearrange("(d p) c -> p d c", p=P),
        )
        nc.sync.dma_start(
            wk_sb[:].rearrange("p (d c) -> p d c", d=DM_T),
            wk[:, hp * P : (hp + 1) * P].rearrange("(d p) c -> p d c", p=P),
        )
        qT = qk_pool.tile([P, SEQ], BF16, tag="qT", name=f"qT{hp}")
        kT = qk_pool.tile([P, SEQ], BF16, tag="kT", name=f"kT{hp}")
        return dict(wq_sb=wq_sb, wk_sb=wk_sb, qT=qT, kT=kT, hp=hp)

    def qk_chunks(pair):
        # k chunk n before q chunk n; chunk n is needed when attention
        # reaches q-chunk n of this pair
        for n in range(N_QC):
            for wsb, dst in (
                (pair["wk_sb"], pair["kT"]),
                (pair["wq_sb"], pair["qT"]),
            ):
                ps = psum_mm.tile(
                    [P, QC], F32, tag="mm", name=f"qkps_{pair['hp']}_{n}"
                )
                for d in range(DM_T):
                    mm = nc.tensor.matmul(
                        ps[:],
                        wsb[:, d * P : (d + 1) * P],
                        xt[:, d * SEQ + n * QC : d * SEQ + (n + 1) * QC],
                        start=(d == 0),
                        stop=(d == DM_T - 1),
                    )
                    pin(mm)
                    if d % 2 == 1 and d < DM_T - 1:
                        yield
                nc.scalar.copy(dst[:, n * QC : (n + 1) * QC], ps[:])
                yield

    # ===== filler queue ===================================================
    fillers = deque()  # (key, generator) — advanced one unit at a time

    def advance_filler():
        while fillers:
            key, gen = fillers[0]
            try:
                next(gen)
                return True
            except StopIteration:
                fillers.popleft()
        return False

    def finish_filler(want_key):
        for key, gen in list(fillers):
            if key == want_key:
                for _ in gen:
                    pass
                fillers.remove((key, gen))

    def v_rest_gen():
        for s in range(12, SEQ_T):
            for _ in v_mms(s):
                yield

    def norm_gen(hp, qc, dcol):
        # 1/d on DVE (no ScalarE table thrash): rows 0/32 hold the two
        # heads' denominators; one K=33 matmul against the 0/1 selector
        # broadcasts both reciprocal rows over the pair's 128 partitions.
        # Unit 1 is DVE-only so the pinned PE chain never waits on it.
        rcol = rcol_pool.tile([P, QC], F32, tag="rcol", name=f"rc_{hp}_{qc}")
        nc.vector.reciprocal_approx_fast(rcol[0:HD, :], dcol[0:HD, :])
        yield
        rc = psum_mm.tile([P, QC], F32, tag="mm", name=f"rcb_{hp}_{qc}")
        mm = nc.tensor.matmul(
            rc[:], sel64[0:HD, :], rcol[0:HD, :], start=True, stop=True
        )
        pin(mm)
        sl = oT[:, hp * SEQ + qc * QC : hp * SEQ + (qc + 1) * QC]
        nc.vector.tensor_mul(sl, sl, rc[:])
        yield

    def proj_gen(s):
        ost = ostage_pool.tile([P, DM], F32, tag="ost", name=f"ost_{s}")
        for n2 in range(2):
            ps = psum_mm.tile([P, QC], F32, tag="mm", name=f"pps_{s}_{n2}")
            for c in range(4):
                mm = nc.tensor.matmul(
                    ps[:],
                    oT[:, c * SEQ + s * P : c * SEQ + (s + 1) * P],
                    wp_sb[:, c * DM + n2 * QC : c * DM + (n2 + 1) * QC],
                    start=(c == 0),
                    stop=(c == 3),
                )
                pin(mm)
                if c == 1:
                    yield
            nc.vector.tensor_copy(ost[:, n2 * QC : (n2 + 1) * QC], ps[:])
            yield
        nc.sync.dma_start(out[s * P : (s + 1) * P, :], ost[:])

    # ===== attention ======================================================
    av_fifo = deque()
    po_cur = {}
    dcol_cur = [None]

    def emit_av_step():
        if not av_fifo:
            return
        rec = av_fifo.popleft()
        hp, qc, nkt, pts = rec["hp"], rec["qc"], rec["nkt"], rec["pts"]
        for idx, kt in enumerate(rec["kts"]):
            for hh in range(2):
                if kt == 0:
                    po_cur[hh] = psum_acc.tile(
                        [P, QC], F32, tag="po", name=f"po_{hp}_{qc}_{hh}"
                    )
                po = po_cur[hh]
                mm = nc.tensor.matmul(
                    po[0 : HD + 1, :],
                    vaug_v[:, kt, 2 * hp + hh, :],
                    pts[hh][:, idx * QC : (idx + 1) * QC],
                    start=(kt == 0),
                    stop=(kt == nkt - 1),
                )
                pin(mm)
                if kt == nkt - 1:
                    if hh == 0:
                        dcol_cur[0] = dcol_pool.tile(
                            [P, QC], F32, tag="dcol", name=f"dc_{hp}_{qc}"
                        )
                        # rows 1-31 must be finite for the [0:33] reciprocal
                        nc.vector.memset(dcol_cur[0][0:HD, :], 1.0)
                    dcol = dcol_cur[0]
                    nc.vector.tensor_copy(
                        dcol[32 * hh : 32 * hh + 1, :], po[HD : HD + 1, :]
                    )
                    nc.vector.tensor_copy(
                        oT[
                            hh * HD : (hh + 1) * HD,
                            hp * SEQ + qc * QC : hp * SEQ + (qc + 1) * QC,
                        ],
                        po[0:HD, :],
                    )
                    if hh == 1:
                        fillers.appendleft(
                            ("norm", norm_gen(hp, qc, dcol))
                        )
                        if hp == NP - 1:
                            for s in range(4 * qc, 4 * qc + 4):
                                fillers.append(("proj", proj_gen(s)))

    pair_cur = qk_pair_dma(0)
    for _ in qk_chunks(pair_cur):  # pair 0 fully upfront
        pass

    budget = {0: 4, 1: 3, 2: 3, 3: 3}
    for hp in range(NP):
        if hp == 0:
            fillers.append(("vrest", v_rest_gen()))
        if hp < NP - 1:
            pair_nxt = qk_pair_dma(hp + 1)
            fillers.append((("qk", hp + 1), qk_chunks(pair_nxt)))
        qT, kT = pair_cur["qT"], pair_cur["kT"]
        for qc in range(N_QC):
            nkt = 4 * qc + 4
            for j in range(nkt // 2):
                k0 = 2 * j
                A = ps_s.tile([P, 2 * QC], F32, tag="s2",
                              name=f"sA_{hp}_{qc}_{j}")
                B = ps_s.tile([P, 2 * QC], F32, tag="s2",
                              name=f"sB_{hp}_{qc}_{j}")
                tiles = {0: A, 1: B}
                for idx, kt in enumerate((k0, k0 + 1)):
                    for hh in range(2):
                        b = hh * HD
                        mm = nc.tensor.matmul(
                            tiles[hh][:, idx * QC : (idx + 1) * QC],
                            kT[b : b + HD, kt * P : (kt + 1) * P],
                            qT[b : b + HD, qc * QC : (qc + 1) * QC],
                            start=True,
                            stop=True,
                        )
                        pin(mm)
                pts = {}
                diag = k0 >= 4 * qc
                for hh in range(2):
                    pt = pt_pool.tile([P, 2 * QC], BF16, tag="pt",
                                      name=f"pt_{hp}_{qc}_{j}_{hh}")
                    pts[hh] = pt
                    # causal fixups run on the otherwise-idle GpSimd engine
                    # to keep the DVE queue short (po/oT drains latency)
                    if not diag:
                        nc.scalar.activation(
                            pt[:], tiles[hh][:], AF.Exp, scale=RSQRT
                        )
                    else:
                        r0 = k0 - 4 * qc
                        off0, off1 = r0 * P, (r0 + 1) * P
                        nc.scalar.activation(
                            pt[:, off0 : 2 * QC],
                            tiles[hh][:, off0 : 2 * QC],
                            AF.Exp,
                            scale=RSQRT,
                        )
                        if off0 > 0:
                            nc.gpsimd.tensor_copy(
                                pt[:, 0:off0], zeros512[:, 0:off0]
                            )
                        nc.gpsimd.tensor_copy(
                            pt[:, QC : QC + off1], zeros512[:, 0:off1]
                        )
                        nc.gpsimd.tensor_mul(
                            pt[:, off0 : off0 + P],
                            pt[:, off0 : off0 + P],
                            mask01[:],
                        )
                        nc.gpsimd.tensor_mul(
                            pt[:, QC + off1 : QC + off1 + P],
                            pt[:, QC + off1 : QC + off1 + P],
                            mask01[:],
                        )
                av_fifo.append(
                    dict(hp=hp, qc=qc, nkt=nkt, pts=pts, kts=(k0, k0 + 1))
                )
                for _ in range(budget[hp]):
                    advance_filler()
                if len(av_fifo) >= 2:  # AV trails S by 2 steps (exp latency)
                    emit_av_step()
        if hp < NP - 1:
            finish_filler(("qk", hp + 1))  # next pair's QK must be complete
            pair_cur = pair_nxt
    while av_fifo:  # drain trailing AV steps
        emit_av_step()
    while advance_filler():  # norms + remaining proj
        pass

    ostage_pool.release()
    rcol_pool.release()
    dcol_pool.release()
    pt_pool.release()
    wqk_pool.release()
    qk_pool.release()
    wp_pool.release()
    oT_pool.release()
    vaug_pool.release()
    wv_pool.release()
    xt_pool.release()
    psum_acc.release()
    psum_mm.release()
    ps_s.release()
    const_pool.release()


_NC_CACHE = None


def _get_program():
    global _NC_CACHE
    if _NC_CACHE is None:
        _NC_CACHE = _build_core_program()
    return _NC_CACHE


BF = ml_dtypes.bfloat16


def _make_in_maps(x, w_qkv, w_proj):
    x = np.asarray(x, dtype=np.float32)
    w_qkv = np.asarray(w_qkv, dtype=np.float32)
    w_proj = np.asarray(w_proj, dtype=np.float32)
    in_maps = []
    for core in range(N_CORES):
        b, g = core // 2, core % 2
        cs = slice(g * COLS, (g + 1) * COLS)
        sel33 = np.zeros((HD, P), dtype=np.float32)
        sel33[0, 0:HD] = 1.0
        sel33[32, HD:P] = 1.0
        in_maps.append(
            {
                "xT": np.ascontiguousarray(x[b].T).astype(BF),
                "sel": sel33,
                "wq": np.ascontiguousarray(
                    w_qkv[:, 0 * DM : 1 * DM][:, cs]
                ).astype(BF),
                "wk": np.ascontiguousarray(
                    w_qkv[:, 1 * DM : 2 * DM][:, cs]
                ).astype(BF),
                "wv": np.ascontiguousarray(
                    w_qkv[:, 2 * DM : 3 * DM][:, cs]
                ).astype(BF),
                "wp": np.ascontiguousarray(w_proj[cs, :]).astype(BF),
            }
        )
    return in_maps


def run_on_hw(x, w_qkv, w_proj, trace=False, **kwargs):
    """Run the SPMD program on 8 cores; returns (full_output, BassKernelResults)."""
    nc = _get_program()
    in_maps = _make_in_maps(x, w_qkv, w_proj)
    res = run_bass_kernel_spmd(
        nc, in_maps, list(range(N_CORES)), trace=trace, **kwargs
    )
    bs = 4
    outp = np.empty((bs, SEQ, DM), dtype=np.float32)
    for b in range(bs):
        outp[b] = res.results[2 * b]["out"] + res.results[2 * b + 1]["out"]
    return outp, res


def kernel(x, w_qkv, w_proj):
    outp, _ = run_on_hw(x, w_qkv, w_proj, trace=False)
    return outp


# revision 31
# speedup vs baseline: 1.0610x; 1.0610x over previous
"""Causal multi-head self-attention on 8 trn2 NeuronCores.

Sharding: 8 cores = 4 batch x 2 head-groups. Core i handles batch i//2 and
heads (i%2)*8 .. (i%2)*8+8 (8 of 16 heads, 512 of 1024 d_model columns).
Each core computes a full (2048, 1024) partial output (its head group pushed
through its w_proj row-slice); the host sums the two partials per batch
element (the tensor-parallel all-reduce done host-side).

All inputs are converted to bf16 on the host; x is host-transposed so x^T
DMAs straight into SBUF (no PE transposes, no staging casts). Per-core
dataflow, everything in transposed layout:
  Q^T, K^T   : w_q/w_k stationary, x^T moving  -> [cols, seq] bf16
  V_aug      : x^T stationary, w_v moving      -> natural [seq, cols] bf16
               + a ones-column per head so the softmax denominator rides the
               AV matmul as output row 64
  S^T        : K^T stationary, Q^T moving; the two heads of a pair go to PE
               row groups 0-63 / 64-127 (K=64) and run concurrently; two
               k-tiles of S land in one 2-bank PSUM tile [128, 1024]
  P^T        : one exp per (head, kt-pair) over the 2-bank tile on ScalarE
               (no max-subtraction: |S|*rsqrt < ~10); causal = skip k>q
               chunks, zero invalid prefixes, 0/1 mask mul on diag blocks
  O^T_aug    : V_aug stationary, P^T moving, accumulated over k-tiles in PSUM
  normalize  : denominators for (h0, h1) of a q-chunk drain to two SBUF
               partitions; 1/d via one reciprocal_approx_fast (DVE), then a
               single K=2 matmul against a 0/1 selector broadcasts both
               reciprocal rows over the pair's 128 partitions; one DVE mul
  out        : O^T stationary, w_proj rows moving -> natural [seq, 1024] f32

Schedule: one strictly-ordered PE chain (dep edges pin every matmul). Per
kt-pair step: 4 S matmuls (pairs packed via row groups), filler units
(next pair's QKV chunks, V tail, normalizes, and for the last head pair the
projection s-tiles), then the previous step's 4 AV matmuls. ScalarE chews
exp one step behind the S matmuls; AV trails exp by a step.
"""

import numpy as np
import ml_dtypes

import concourse.bass as bass
import concourse.mybir as mybir
import concourse.tile as tile
from concourse import bacc
from concourse.bass_utils import run_bass_kernel_spmd
from concourse.masks import make_upper_triangular
from concourse.tile_rust import add_dep_helper
from collections import deque

F32 = mybir.dt.float32
BF16 = mybir.dt.bfloat16
AF = mybir.ActivationFunctionType

SEQ = 2048
DM = 1024
COLS = 512          # head-cols per core (8 heads x 64)
HD = 64
P = 128
N_CORES = 8
RSQRT = 0.125       # 1/sqrt(64)

SEQ_T = SEQ // P    # 16 seq tiles
DM_T = DM // P      # 8 d_model tiles
QC = 512            # q-chunk (PSUM free size)
N_QC = SEQ // QC    # 4 q chunks
NP = 4              # head pairs per core


def _build_core_program():
    nc = bacc.Bacc(
        "TRN2", target_bir_lowering=False, debug=False, num_devices=N_CORES
    )
    xT = nc.dram_tensor("xT", [DM, SEQ], BF16, kind="ExternalInput").ap()
    sel = nc.dram_tensor("sel", [HD, P], F32, kind="ExternalInput").ap()
    wq = nc.dram_tensor("wq", [DM, COLS], BF16, kind="ExternalInput").ap()
    wk = nc.dram_tensor("wk", [DM, COLS], BF16, kind="ExternalInput").ap()
    wv = nc.dram_tensor("wv", [DM, COLS], BF16, kind="ExternalInput").ap()
    wp = nc.dram_tensor("wp", [COLS, DM], BF16, kind="ExternalInput").ap()
    out = nc.dram_tensor("out", [SEQ, DM], F32, kind="ExternalOutput").ap()

    with tile.TileContext(nc) as tc:
        _emit(tc, xT, sel, wq, wk, wv, wp, out)
    nc.compile()
    return nc


def _emit(tc, xT, sel, wq, wk, wv, wp, out):
    nc = tc.nc

    # strict PE order: every matmul chains onto the previous one (order-only
    # edge, no semaphore) so the scheduler cannot interpose PE work between
    # an S row-group pair, which would break their concurrent execution
    chain = [None]

    def pin(mm):
        if chain[0] is not None:
            add_dep_helper(mm.ins, chain[0], sync=False, reason="pe-chain")
        chain[0] = mm.ins

    # --- pools ------------------------------------------------------------
    const_pool = tc.alloc_tile_pool(name="const", bufs=1)
    ps_s = tc.alloc_tile_pool(name="ps_s", bufs=2, space="PSUM")      # 4 banks
    psum_mm = tc.alloc_tile_pool(name="psum_mm", bufs=2, space="PSUM")  # 2
    psum_acc = tc.alloc_tile_pool(name="psum_acc", bufs=2, space="PSUM")  # 2

    # --- constants --------------------------------------------------------
    mask01 = const_pool.tile([P, P], BF16, tag="mask01")
    # 1.0 where free-idx (q) >= partition-idx (k), else 0 — causal in S^T
    make_upper_triangular(nc, mask01[:], val=1.0, diag=True)
    cstage = const_pool.tile([P, QC], F32, tag="cstage")
    nc.vector.memset(cstage[:], 0.0)
    zeros512 = const_pool.tile([P, QC], BF16, tag="zeros512")
    nc.vector.tensor_copy(zeros512[:], cstage[:])
    nc.vector.memset(cstage[:, 0:8], 1.0)
    ones8 = const_pool.tile([P, 8], BF16, tag="ones8")
    nc.vector.tensor_copy(ones8[:], cstage[:, 0:8])
    # 0/1 selector for the reciprocal broadcast (host-built): contraction row
    # 0 carries h0's reciprocal to out partitions 0-63, row 32 carries h1's
    # to 64-127. Staged to f32r — plain-fp32 matmuls split into two HI/LO
    # passes at half rate.
    F32R = mybir.dt.float32r
    sel_stage = const_pool.tile([HD, P], F32, tag="sel_stage")
    nc.sync.dma_start(sel_stage[:], sel[:, :])
    sel64 = const_pool.tile([HD, P], F32R, tag="sel64")
    nc.vector.tensor_copy(sel64[:], sel_stage[:])

    # --- persistent SBUF --------------------------------------------------
    xt_pool = tc.alloc_tile_pool(name="xt", bufs=1)
    xt = xt_pool.tile([P, DM_T * SEQ], BF16, tag="xt")  # x^T, d-tile major
    wv_pool = tc.alloc_tile_pool(name="wv", bufs=1)
    wv_sb = wv_pool.tile([P, DM_T * COLS], BF16, tag="wv_sb")
    vaug_pool = tc.alloc_tile_pool(name="vaug", bufs=1)
    vaug = vaug_pool.tile([P, SEQ_T * 8 * (HD + 1)], BF16, tag="vaug")
    vaug_v = vaug[:].rearrange("p (s h e) -> p s h e", s=SEQ_T, h=8)
    oT_pool = tc.alloc_tile_pool(name="oT", bufs=1)
    oT = oT_pool.tile([P, NP * SEQ], BF16, tag="oT")
    wp_pool = tc.alloc_tile_pool(name="wp", bufs=1)
    wp_sb = wp_pool.tile([P, 4 * DM], BF16, tag="wp_sb")

    qk_pool = tc.alloc_tile_pool(name="qk", bufs=2)
    wqk_pool = tc.alloc_tile_pool(name="wqk", bufs=2)
    pt_pool = tc.alloc_tile_pool(name="pt", bufs=6)
    dcol_pool = tc.alloc_tile_pool(name="dcol", bufs=2)
    rcol_pool = tc.alloc_tile_pool(name="rcol", bufs=2)
    ostage_pool = tc.alloc_tile_pool(name="ostage", bufs=3)

    # --- input DMAs (bf16, direct into final layout) ----------------------
    for d in range(DM_T):
        nc.sync.dma_start(
            xt[:, d * SEQ : (d + 1) * SEQ], xT[d * P : (d + 1) * P, :]
        )
        nc.sync.dma_start(
            wv_sb[:, d * COLS : (d + 1) * COLS], wv[d * P : (d + 1) * P, :]
        )
    for c in range(4):
        nc.sync.dma_start(
            wp_sb[:, c * DM : (c + 1) * DM], wp[c * P : (c + 1) * P, :]
        )

    # ===== V (natural layout) + ones columns ==============================
    def v_mms(s):
        ps = psum_mm.tile([P, QC], F32, tag="mm", name=f"vps_{s}")
        for d in range(DM_T):
            mm = nc.tensor.matmul(
                ps[:],
                xt[:, d * SEQ + s * P : d * SEQ + (s + 1) * P],
                wv_sb[:, d * COLS : (d + 1) * COLS],
                start=(d == 0),
                stop=(d == DM_T - 1),
            )
            pin(mm)
            if d % 2 == 1 and d < DM_T - 1:
                yield
        nc.vector.tensor_copy(
            vaug_v[:, s, :, 0:HD], ps[:].rearrange("p (h e) -> p h e", h=8)
        )
        nc.vector.tensor_copy(
            vaug_v[:, s, :, HD : HD + 1],
            ones8[:].rearrange("p (a b) -> p a b", b=1),
        )
        yield

    for s in range(12):  # first 12 seq-tiles upfront; rest as fillers
        for _ in v_mms(s):
            pass

    # ===== Q^T / K^T pair machinery =======================================
    def qk_pair_dma(hp):
        wq_sb = wqk_pool.tile([P, DM_T * P], BF16, tag="wq_sb", name=f"wq{hp}")
        wk_sb = wqk_pool.tile([P, DM_T * P], BF16, tag="wk_sb", name=f"wk{hp}")
        nc.sync.dma_start(
            wq_sb[:].rearrange("p (d c) -> p d c", d=DM_T),
            wq[:, hp * P : (hp + 1) * P].rearrange("(d p) c -> p d c", p=P),
        )
        nc.sync.dma_start(
            wk_sb[:].rearrange("p (d c) -> p d c", d=DM_T),
            wk[:, hp * P : (hp + 1) * P].rearrange("(d p) c -> p d c", p=P),
        )
        qT = qk_pool.tile([P, SEQ], BF16, tag="qT", name=f"qT{hp}")
        kT = qk_pool.tile([P, SEQ], BF16, tag="kT", name=f"kT{hp}")
        return dict(wq_sb=wq_sb, wk_sb=wk_sb, qT=qT, kT=kT, hp=hp)

    def qk_chunks(pair):
        # k chunk n before q chunk n; chunk n is needed when attention
        # reaches q-chunk n of this pair
        for n in range(N_QC):
            for wsb, dst in (
                (pair["wk_sb"], pair["kT"]),
                (pair["wq_sb"], pair["qT"]),
            ):
                ps = psum_mm.tile(
                    [P, QC], F32, tag="mm", name=f"qkps_{pair['hp']}_{n}"
                )
                for d in range(DM_T):
                    mm = nc.tensor.matmul(
                        ps[:],
                        wsb[:, d * P : (d + 1) * P],
                        xt[:, d * SEQ + n * QC : d * SEQ + (n + 1) * QC],
                        start=(d == 0),
                        stop=(d == DM_T - 1),
                    )
                    pin(mm)
                    if d % 2 == 1 and d < DM_T - 1:
                        yield
                nc.vector.tensor_copy(dst[:, n * QC : (n + 1) * QC], ps[:])
                yield

    # ===== filler queue ===================================================
    fillers = deque()  # (key, generator) — advanced one unit at a time

    def advance_filler():
        while fillers:
            key, gen = fillers[0]
            try:
                next(gen)
                return True
            except StopIteration:
                fillers.popleft()
        return False

    def finish_filler(want_key):
        for key, gen in list(fillers):
            if key == want_key:
                for _ in gen:
                    pass
                fillers.remove((key, gen))

    def v_rest_gen():
        for s in range(12, SEQ_T):
            for _ in v_mms(s):
                yield

    def norm_gen(hp, qc, dcol):
        # 1/d on DVE (no ScalarE table thrash): rows 0/32 hold the two
        # heads' denominators; one K=33 matmul against the 0/1 selector
        # broadcasts both reciprocal rows over the pair's 128 partitions.
        # Unit 1 is DVE-only so the pinned PE chain never waits on it.
        rcol = rcol_pool.tile([P, QC], F32, tag="rcol", name=f"rc_{hp}_{qc}")
        rcr = rcol_pool.tile([P, QC], mybir.dt.float32r, tag="rcr",
                             name=f"rcr_{hp}_{qc}")
        nc.vector.reciprocal_approx_fast(rcol[0:HD, :], dcol[0:HD, :])
        nc.vector.tensor_copy(rcr[0:HD, :], rcol[0:HD, :])
        yield
        rc = psum_mm.tile([P, QC], F32, tag="mm", name=f"rcb_{hp}_{qc}")
        mm = nc.tensor.matmul(
            rc[:], sel64[0:HD, :], rcr[0:HD, :], start=True, stop=True
        )
        pin(mm)
        sl = oT[:, hp * SEQ + qc * QC : hp * SEQ + (qc + 1) * QC]
        nc.vector.tensor_mul(sl, sl, rc[:])
        yield

    def proj_gen(s):
        ost = ostage_pool.tile([P, DM], F32, tag="ost", name=f"ost_{s}")
        for n2 in range(2):
            ps = psum_mm.tile([P, QC], F32, tag="mm", name=f"pps_{s}_{n2}")
            for c in range(4):
                mm = nc.tensor.matmul(
                    ps[:],
                    oT[:, c * SEQ + s * P : c * SEQ + (s + 1) * P],
                    wp_sb[:, c * DM + n2 * QC : c * DM + (n2 + 1) * QC],
                    start=(c == 0),
                    stop=(c == 3),
                )
                pin(mm)
                if c == 1:
                    yield
            nc.vector.tensor_copy(ost[:, n2 * QC : (n2 + 1) * QC], ps[:])
            yield
        nc.sync.dma_start(out[s * P : (s + 1) * P, :], ost[:])

    # ===== attention ======================================================
    av_fifo = deque()
    po_cur = {}
    dcol_cur = [None]

    def emit_av_step():
        if not av_fifo:
            return
        rec = av_fifo.popleft()
        hp, qc, nkt, pts = rec["hp"], rec["qc"], rec["nkt"], rec["pts"]
        for idx, kt in enumerate(rec["kts"]):
            for hh in range(2):
                if kt == 0:
                    po_cur[hh] = psum_acc.tile(
                        [P, QC], F32, tag="po", name=f"po_{hp}_{qc}_{hh}"
                    )
                po = po_cur[hh]
                mm = nc.tensor.matmul(
                    po[0 : HD + 1, :],
                    vaug_v[:, kt, 2 * hp + hh, :],
                    pts[hh][:, idx * QC : (idx + 1) * QC],
                    start=(kt == 0),
                    stop=(kt == nkt - 1),
                )
                pin(mm)
                if kt == nkt - 1:
                    if hh == 0:
                        dcol_cur[0] = dcol_pool.tile(
                            [P, QC], F32, tag="dcol", name=f"dc_{hp}_{qc}"
                        )
                        # rows 1-31 must be finite for the [0:33] reciprocal
                        nc.vector.memset(dcol_cur[0][0:HD, :], 1.0)
                    dcol = dcol_cur[0]
                    nc.vector.tensor_copy(
                        dcol[32 * hh : 32 * hh + 1, :], po[HD : HD + 1, :]
                    )
                    nc.vector.tensor_copy(
                        oT[
                            hh * HD : (hh + 1) * HD,
                            hp * SEQ + qc * QC : hp * SEQ + (qc + 1) * QC,
                        ],
                        po[0:HD, :],
                    )
                    if hh == 1:
                        fillers.appendleft(
                            ("norm", norm_gen(hp, qc, dcol))
                        )
                        if hp == NP - 1:
                            for s in range(4 * qc, 4 * qc + 4):
                                fillers.append(("proj", proj_gen(s)))

    pair_cur = qk_pair_dma(0)
    for _ in qk_chunks(pair_cur):  # pair 0 fully upfront
        pass

    budget = {0: 4, 1: 3, 2: 3, 3: 3}
    for hp in range(NP):
        if hp == 0:
            fillers.append(("vrest", v_rest_gen()))
        if hp < NP - 1:
            pair_nxt = qk_pair_dma(hp + 1)
            fillers.append((("qk", hp + 1), qk_chunks(pair_nxt)))
        qT, kT = pair_cur["qT"], pair_cur["kT"]
        for qc in range(N_QC):
            nkt = 4 * qc + 4
            for j in range(nkt // 2):
                k0 = 2 * j
                A = ps_s.tile([P, 2 * QC], F32, tag="s2",
                              name=f"sA_{hp}_{qc}_{j}")
                B = ps_s.tile([P, 2 * QC], F32, tag="s2",
                              name=f"sB_{hp}_{qc}_{j}")
                tiles = {0: A, 1: B}
                for idx, kt in enumerate((k0, k0 + 1)):
                    for hh in range(2):
                        b = hh * HD
                        mm = nc.tensor.matmul(
                            tiles[hh][:, idx * QC : (idx + 1) * QC],
                            kT[b : b + HD, kt * P : (kt + 1) * P],
                            qT[b : b + HD, qc * QC : (qc + 1) * QC],
                            start=True,
                            stop=True,
                        )
                        pin(mm)
                pts = {}
                diag = k0 >= 4 * qc
                for hh in range(2):
                    pt = pt_pool.tile([P, 2 * QC], BF16, tag="pt",
                                      name=f"pt_{hp}_{qc}_{j}_{hh}")
                    pts[hh] = pt
                    if not diag:
                        nc.scalar.activation(
                            pt[:], tiles[hh][:], AF.Exp, scale=RSQRT
                        )
                    else:
                        r0 = k0 - 4 * qc
                        off0, off1 = r0 * P, (r0 + 1) * P
                        nc.scalar.activation(
                            pt[:, off0 : 2 * QC],
                            tiles[hh][:, off0 : 2 * QC],
                            AF.Exp,
                            scale=RSQRT,
                        )
                        if off0 > 0:
                            nc.vector.tensor_copy(
                                pt[:, 0:off0], zeros512[:, 0:off0]
                            )
                        nc.vector.tensor_copy(
                            pt[:, QC : QC + off1], zeros512[:, 0:off1]
                        )
                        nc.vector.tensor_mul(
                            pt[:, off0 : off0 + P],
                            pt[:, off0 : off0 + P],
                            mask01[:],
                        )
                        nc.vector.tensor_mul(
                            pt[:, QC + off1 : QC + off1 + P],
                            pt[:, QC + off1 : QC + off1 + P],
                            mask01[:],
                        )
                av_fifo.append(
                    dict(hp=hp, qc=qc, nkt=nkt, pts=pts, kts=(k0, k0 + 1))
                )
                if len(av_fifo) > 2:  # AV trails S by 2 steps (exp latency)
                    emit_av_step()
                for _ in range(budget[hp]):
                    advance_filler()
        if hp < NP - 1:
            finish_filler(("qk", hp + 1))  # next pair's QK must be complete
            pair_cur = pair_nxt
    while av_fifo:  # drain trailing AV steps
        emit_av_step()
    while advance_filler():  # norms + remaining proj
        pass

    ostage_pool.release()
    rcol_pool.release()
    dcol_pool.release()
    pt_pool.release()
    wqk_pool.release()
    qk_pool.release()
    wp_pool.release()
    oT_pool.release()
    vaug_pool.release()
    wv_pool.release()
    xt_pool.release()
    psum_acc.release()
    psum_mm.release()
    ps_s.release()
    const_pool.release()


_NC_CACHE = None


def _get_program():
    global _NC_CACHE
    if _NC_CACHE is None:
        _NC_CACHE = _build_core_program()
    return _NC_CACHE


BF = ml_dtypes.bfloat16


def _make_in_maps(x, w_qkv, w_proj):
    x = np.asarray(x, dtype=np.float32)
    w_qkv = np.asarray(w_qkv, dtype=np.float32)
    w_proj = np.asarray(w_proj, dtype=np.float32)
    in_maps = []
    for core in range(N_CORES):
        b, g = core // 2, core % 2
        cs = slice(g * COLS, (g + 1) * COLS)
        sel33 = np.zeros((HD, P), dtype=np.float32)
        sel33[0, 0:HD] = 1.0
        sel33[32, HD:P] = 1.0
        in_maps.append(
            {
                "xT": np.ascontiguousarray(x[b].T).astype(BF),
                "sel": sel33,
                "wq": np.ascontiguousarray(
                    w_qkv[:, 0 * DM : 1 * DM][:, cs]
                ).astype(BF),
                "wk": np.ascontiguousarray(
                    w_qkv[:, 1 * DM : 2 * DM][:, cs]
                ).astype(BF),
                "wv": np.ascontiguousarray(
                    w_qkv[:, 2 * DM : 3 * DM][:, cs]
                ).astype(BF),
                "wp": np.ascontiguousarray(w_proj[cs, :]).astype(BF),
            }
        )
    return in_maps


def run_on_hw(x, w_qkv, w_proj, trace=False, **kwargs):
    """Run the SPMD program on 8 cores; returns (full_output, BassKernelResults)."""
    nc = _get_program()
    in_maps = _make_in_maps(x, w_qkv, w_proj)
    res = run_bass_kernel_spmd(
        nc, in_maps, list(range(N_CORES)), trace=trace, **kwargs
    )
    bs = 4
    outp = np.empty((bs, SEQ, DM), dtype=np.float32)
    for b in range(bs):
        outp[b] = res.results[2 * b]["out"] + res.results[2 * b + 1]["out"]
    return outp, res


def kernel(x, w_qkv, w_proj):
    outp, _ = run_on_hw(x, w_qkv, w_proj, trace=False)
    return outp


# revision 33
# speedup vs baseline: 1.2283x; 1.1577x over previous
"""Causal multi-head self-attention on 8 trn2 NeuronCores.

Sharding: 8 cores = 4 batch x 2 head-groups. Core i handles batch i//2 and
heads (i%2)*8 .. (i%2)*8+8 (8 of 16 heads, 512 of 1024 d_model columns).
Each core computes a full (2048, 1024) partial output (its head group pushed
through its w_proj row-slice); the host sums the two partials per batch
element (the tensor-parallel all-reduce done host-side).

All inputs are converted to bf16 on the host; x is host-transposed so x^T
DMAs straight into SBUF (no PE transposes, no staging casts). Per-core
dataflow, everything in transposed layout:
  Q^T, K^T   : w_q/w_k stationary, x^T moving  -> [cols, seq] bf16
  V_aug      : x^T stationary, w_v moving      -> natural [seq, cols] bf16
               + a ones-column per head so the softmax denominator rides the
               AV matmul as output row 64
  S^T        : K^T stationary, Q^T moving; the two heads of a pair go to PE
               row groups 0-63 / 64-127 (K=64) and run concurrently; two
               k-tiles of S land in one 2-bank PSUM tile [128, 1024]
  P^T        : one exp per (head, kt-pair) over the 2-bank tile on ScalarE
               (no max-subtraction: |S|*rsqrt < ~10); causal = skip k>q
               chunks, zero invalid prefixes, 0/1 mask mul on diag blocks
  O^T_aug    : V_aug stationary, P^T moving, accumulated over k-tiles in PSUM
  normalize  : denominators for (h0, h1) of a q-chunk drain to two SBUF
               partitions; 1/d via one reciprocal_approx_fast (DVE), then a
               single K=2 matmul against a 0/1 selector broadcasts both
               reciprocal rows over the pair's 128 partitions; one DVE mul
  out        : O^T stationary, w_proj rows moving -> natural [seq, 1024] f32

Schedule: one strictly-ordered PE chain (dep edges pin every matmul). Per
kt-pair step: 4 S matmuls (pairs packed via row groups), filler units
(next pair's QKV chunks, V tail, normalizes, and for the last head pair the
projection s-tiles), then the previous step's 4 AV matmuls. ScalarE chews
exp one step behind the S matmuls; AV trails exp by a step.
"""

import numpy as np
import ml_dtypes

import concourse.bass as bass
import concourse.mybir as mybir
import concourse.tile as tile
from concourse import bacc
from concourse.bass_utils import run_bass_kernel_spmd
from concourse.masks import make_upper_triangular
from concourse.tile_rust import add_dep_helper
from collections import deque

F32 = mybir.dt.float32
BF16 = mybir.dt.bfloat16
AF = mybir.ActivationFunctionType

SEQ = 2048
DM = 1024
COLS = 512          # head-cols per core (8 heads x 64)
HD = 64
P = 128
N_CORES = 8
RSQRT = 0.125       # 1/sqrt(64)

SEQ_T = SEQ // P    # 16 seq tiles
DM_T = DM // P      # 8 d_model tiles
QC = 512            # q-chunk (PSUM free size)
N_QC = SEQ // QC    # 4 q chunks
NP = 4              # head pairs per core


def _build_core_program():
    nc = bacc.Bacc(
        "TRN2", target_bir_lowering=False, debug=False, num_devices=N_CORES
    )
    xT = nc.dram_tensor("xT", [DM, SEQ], BF16, kind="ExternalInput").ap()
    sel = nc.dram_tensor("sel", [HD, P], F32, kind="ExternalInput").ap()
    wq = nc.dram_tensor("wq", [DM, COLS], BF16, kind="ExternalInput").ap()
    wk = nc.dram_tensor("wk", [DM, COLS], BF16, kind="ExternalInput").ap()
    wv = nc.dram_tensor("wv", [DM, COLS], BF16, kind="ExternalInput").ap()
    wp = nc.dram_tensor("wp", [COLS, DM], BF16, kind="ExternalInput").ap()
    out = nc.dram_tensor("out", [SEQ, DM], F32, kind="ExternalOutput").ap()

    with tile.TileContext(nc) as tc:
        _emit(tc, xT, sel, wq, wk, wv, wp, out)
    nc.compile()
    return nc


def _emit(tc, xT, sel, wq, wk, wv, wp, out):
    nc = tc.nc

    # strict PE order: every matmul chains onto the previous one (order-only
    # edge, no semaphore) so the scheduler cannot interpose PE work between
    # an S row-group pair, which would break their concurrent execution
    chain = [None]

    def pin(mm):
        if chain[0] is not None:
            add_dep_helper(mm.ins, chain[0], sync=False, reason="pe-chain")
        chain[0] = mm.ins

    # --- pools ------------------------------------------------------------
    const_pool = tc.alloc_tile_pool(name="const", bufs=1)
    ps_s = tc.alloc_tile_pool(name="ps_s", bufs=2, space="PSUM")      # 4 banks
    psum_mm = tc.alloc_tile_pool(name="psum_mm", bufs=2, space="PSUM")  # 2
    psum_acc = tc.alloc_tile_pool(name="psum_acc", bufs=2, space="PSUM")  # 2

    # --- constants --------------------------------------------------------
    mask01 = const_pool.tile([P, P], BF16, tag="mask01")
    # 1.0 where free-idx (q) >= partition-idx (k), else 0 — causal in S^T
    make_upper_triangular(nc, mask01[:], val=1.0, diag=True)
    cstage = const_pool.tile([P, QC], F32, tag="cstage")
    nc.vector.memset(cstage[:], 0.0)
    zeros512 = const_pool.tile([P, QC], BF16, tag="zeros512")
    nc.vector.tensor_copy(zeros512[:], cstage[:])
    nc.vector.memset(cstage[:, 0:8], 1.0)
    ones8 = const_pool.tile([P, 8], BF16, tag="ones8")
    nc.vector.tensor_copy(ones8[:], cstage[:, 0:8])
    # 0/1 selector for the reciprocal broadcast (host-built): contraction row
    # 0 carries h0's reciprocal to out partitions 0-63, row 32 carries h1's
    # to 64-127. Staged to f32r — plain-fp32 matmuls split into two HI/LO
    # passes at half rate.
    F32R = mybir.dt.float32r
    sel_stage = const_pool.tile([HD, P], F32, tag="sel_stage")
    nc.sync.dma_start(sel_stage[:], sel[:, :])
    sel64 = const_pool.tile([HD, P], F32R, tag="sel64")
    nc.vector.tensor_copy(sel64[:], sel_stage[:])

    # --- persistent SBUF --------------------------------------------------
    xt_pool = tc.alloc_tile_pool(name="xt", bufs=1)
    xt = xt_pool.tile([P, DM_T * SEQ], BF16, tag="xt")  # x^T, d-tile major
    wv_pool = tc.alloc_tile_pool(name="wv", bufs=1)
    wv_sb = wv_pool.tile([P, DM_T * COLS], BF16, tag="wv_sb")
    vaug_pool = tc.alloc_tile_pool(name="vaug", bufs=1)
    vaug = vaug_pool.tile([P, SEQ_T * 8 * (HD + 1)], BF16, tag="vaug")
    vaug_v = vaug[:].rearrange("p (s h e) -> p s h e", s=SEQ_T, h=8)
    oT_pool = tc.alloc_tile_pool(name="oT", bufs=1)
    oT = oT_pool.tile([P, NP * SEQ], BF16, tag="oT")
    wp_pool = tc.alloc_tile_pool(name="wp", bufs=1)
    wp_sb = wp_pool.tile([P, 4 * DM], BF16, tag="wp_sb")

    qk_pool = tc.alloc_tile_pool(name="qk", bufs=2)
    wqk_pool = tc.alloc_tile_pool(name="wqk", bufs=2)
    pt_pool = tc.alloc_tile_pool(name="pt", bufs=6)
    dcol_pool = tc.alloc_tile_pool(name="dcol", bufs=4)
    rcol_pool = tc.alloc_tile_pool(name="rcol", bufs=4)
    ostage_pool = tc.alloc_tile_pool(name="ostage", bufs=3)

    # --- input DMAs (bf16, direct into final layout) ----------------------
    for d in range(DM_T):
        nc.sync.dma_start(
            xt[:, d * SEQ : (d + 1) * SEQ], xT[d * P : (d + 1) * P, :]
        )
        nc.sync.dma_start(
            wv_sb[:, d * COLS : (d + 1) * COLS], wv[d * P : (d + 1) * P, :]
        )
    for c in range(4):
        nc.sync.dma_start(
            wp_sb[:, c * DM : (c + 1) * DM], wp[c * P : (c + 1) * P, :]
        )

    # ===== V (natural layout) + ones columns ==============================
    def v_mms(s):
        ps = psum_mm.tile([P, QC], F32, tag="mm", name=f"vps_{s}")
        for d in range(DM_T):
            mm = nc.tensor.matmul(
                ps[:],
                xt[:, d * SEQ + s * P : d * SEQ + (s + 1) * P],
                wv_sb[:, d * COLS : (d + 1) * COLS],
                start=(d == 0),
                stop=(d == DM_T - 1),
            )
            pin(mm)
            if d % 2 == 1 and d < DM_T - 1:
                yield
        nc.vector.tensor_copy(
            vaug_v[:, s, :, 0:HD], ps[:].rearrange("p (h e) -> p h e", h=8)
        )
        nc.vector.tensor_copy(
            vaug_v[:, s, :, HD : HD + 1],
            ones8[:].rearrange("p (a b) -> p a b", b=1),
        )
        yield

    for s in range(12):  # first 12 seq-tiles upfront; rest as fillers
        for _ in v_mms(s):
            pass

    # ===== Q^T / K^T pair machinery =======================================
    def qk_pair_dma(hp):
        wq_sb = wqk_pool.tile([P, DM_T * P], BF16, tag="wq_sb", name=f"wq{hp}")
        wk_sb = wqk_pool.tile([P, DM_T * P], BF16, tag="wk_sb", name=f"wk{hp}")
        nc.sync.dma_start(
            wq_sb[:].rearrange("p (d c) -> p d c", d=DM_T),
            wq[:, hp * P : (hp + 1) * P].rearrange("(d p) c -> p d c", p=P),
        )
        nc.sync.dma_start(
            wk_sb[:].rearrange("p (d c) -> p d c", d=DM_T),
            wk[:, hp * P : (hp + 1) * P].rearrange("(d p) c -> p d c", p=P),
        )
        qT = qk_pool.tile([P, SEQ], BF16, tag="qT", name=f"qT{hp}")
        kT = qk_pool.tile([P, SEQ], BF16, tag="kT", name=f"kT{hp}")
        return dict(wq_sb=wq_sb, wk_sb=wk_sb, qT=qT, kT=kT, hp=hp)

    def qk_chunks(pair):
        # k chunk n before q chunk n; chunk n is needed when attention
        # reaches q-chunk n of this pair
        for n in range(N_QC):
            for wsb, dst in (
                (pair["wk_sb"], pair["kT"]),
                (pair["wq_sb"], pair["qT"]),
            ):
                ps = psum_mm.tile(
                    [P, QC], F32, tag="mm", name=f"qkps_{pair['hp']}_{n}"
                )
                for d in range(DM_T):
                    mm = nc.tensor.matmul(
                        ps[:],
                        wsb[:, d * P : (d + 1) * P],
                        xt[:, d * SEQ + n * QC : d * SEQ + (n + 1) * QC],
                        start=(d == 0),
                        stop=(d == DM_T - 1),
                    )
                    pin(mm)
                    if d % 2 == 1 and d < DM_T - 1:
                        yield
                nc.vector.tensor_copy(dst[:, n * QC : (n + 1) * QC], ps[:])
                yield

    # ===== filler queue ===================================================
    fillers = deque()  # (key, generator) — advanced one unit at a time

    def advance_filler():
        while fillers:
            key, gen = fillers[0]
            try:
                next(gen)
                return True
            except StopIteration:
                fillers.popleft()
        return False

    def finish_filler(want_key):
        for key, gen in list(fillers):
            if key == want_key:
                for _ in gen:
                    pass
                fillers.remove((key, gen))

    def v_rest_gen():
        for s in range(12, SEQ_T):
            for _ in v_mms(s):
                yield

    def norm_gen(hp, qc, dcol):
        # 1/d on DVE (no ScalarE table thrash): rows 0/32 hold the two
        # heads' denominators; one K=33 matmul against the 0/1 selector
        # broadcasts both reciprocal rows over the pair's 128 partitions.
        # Unit 1 is DVE-only so the pinned PE chain never waits on it.
        rcol = rcol_pool.tile([P, QC], F32, tag="rcol", name=f"rc_{hp}_{qc}")
        rcr = rcol_pool.tile([P, QC], mybir.dt.float32r, tag="rcr",
                             name=f"rcr_{hp}_{qc}")
        nc.vector.reciprocal_approx_fast(rcol[0:HD, :], dcol[0:HD, :])
        nc.vector.tensor_copy(rcr[0:HD, :], rcol[0:HD, :])
        yield
        rc = psum_mm.tile([P, QC], F32, tag="mm", name=f"rcb_{hp}_{qc}")
        mm = nc.tensor.matmul(
            rc[:], sel64[0:HD, :], rcr[0:HD, :], start=True, stop=True
        )
        pin(mm)
        sl = oT[:, hp * SEQ + qc * QC : hp * SEQ + (qc + 1) * QC]
        nc.vector.tensor_mul(sl, sl, rc[:])
        yield

    def proj_gen(s):
        ost = ostage_pool.tile([P, DM], F32, tag="ost", name=f"ost_{s}")
        for n2 in range(2):
            ps = psum_mm.tile([P, QC], F32, tag="mm", name=f"pps_{s}_{n2}")
            for c in range(4):
                mm = nc.tensor.matmul(
                    ps[:],
                    oT[:, c * SEQ + s * P : c * SEQ + (s + 1) * P],
                    wp_sb[:, c * DM + n2 * QC : c * DM + (n2 + 1) * QC],
                    start=(c == 0),
                    stop=(c == 3),
                )
                pin(mm)
                if c == 1:
                    yield
            nc.vector.tensor_copy(ost[:, n2 * QC : (n2 + 1) * QC], ps[:])
            yield
        nc.sync.dma_start(out[s * P : (s + 1) * P, :], ost[:])

    # ===== attention ======================================================
    av_fifo = deque()
    po_cur = {}
    dcol_cur = [None]

    def emit_av_step():
        if not av_fifo:
            return
        rec = av_fifo.popleft()
        hp, qc, nkt, pts = rec["hp"], rec["qc"], rec["nkt"], rec["pts"]
        for idx, kt in enumerate(rec["kts"]):
            for hh in range(2):
                if kt == 0:
                    po_cur[hh] = psum_acc.tile(
                        [P, QC], F32, tag="po", name=f"po_{hp}_{qc}_{hh}"
                    )
                po = po_cur[hh]
                mm = nc.tensor.matmul(
                    po[0 : HD + 1, :],
                    vaug_v[:, kt, 2 * hp + hh, :],
                    pts[hh][:, idx * QC : (idx + 1) * QC],
                    start=(kt == 0),
                    stop=(kt == nkt - 1),
                )
                pin(mm)
                if kt == nkt - 1:
                    if hh == 0:
                        dcol_cur[0] = dcol_pool.tile(
                            [P, QC], F32, tag="dcol", name=f"dc_{hp}_{qc}"
                        )
                        # rows 1-31 must be finite for the [0:33] reciprocal
                        nc.vector.memset(dcol_cur[0][0:HD, :], 1.0)
                    dcol = dcol_cur[0]
                    nc.vector.tensor_copy(
                        dcol[32 * hh : 32 * hh + 1, :], po[HD : HD + 1, :]
                    )
                    nc.vector.tensor_copy(
                        oT[
                            hh * HD : (hh + 1) * HD,
                            hp * SEQ + qc * QC : hp * SEQ + (qc + 1) * QC,
                        ],
                        po[0:HD, :],
                    )
                    if hh == 1:
                        # normalize is latency-tolerant (oT slice only read
                        # by proj): queue at the BACK so the chain never
                        # waits on its DVE reciprocal dependency chain
                        fillers.append(("norm", norm_gen(hp, qc, dcol)))
                        if hp == NP - 1:
                            for s in range(4 * qc, 4 * qc + 4):
                                fillers.append(("proj", proj_gen(s)))

    pair_cur = qk_pair_dma(0)
    for _ in qk_chunks(pair_cur):  # pair 0 fully upfront
        pass

    budget = {0: 4, 1: 3, 2: 3, 3: 3}
    for hp in range(NP):
        if hp == 0:
            fillers.append(("vrest", v_rest_gen()))
        if hp < NP - 1:
            pair_nxt = qk_pair_dma(hp + 1)
            fillers.append((("qk", hp + 1), qk_chunks(pair_nxt)))
        qT, kT = pair_cur["qT"], pair_cur["kT"]
        for qc in range(N_QC):
            nkt = 4 * qc + 4
            for j in range(nkt // 2):
                k0 = 2 * j
                A = ps_s.tile([P, 2 * QC], F32, tag="s2",
                              name=f"sA_{hp}_{qc}_{j}")
                B = ps_s.tile([P, 2 * QC], F32, tag="s2",
                              name=f"sB_{hp}_{qc}_{j}")
                tiles = {0: A, 1: B}
                for idx, kt in enumerate((k0, k0 + 1)):
                    for hh in range(2):
                        b = hh * HD
                        mm = nc.tensor.matmul(
                            tiles[hh][:, idx * QC : (idx + 1) * QC],
                            kT[b : b + HD, kt * P : (kt + 1) * P],
                            qT[b : b + HD, qc * QC : (qc + 1) * QC],
                            start=True,
                            stop=True,
                        )
                        pin(mm)
                pts = {}
                diag = k0 >= 4 * qc
                for hh in range(2):
                    pt = pt_pool.tile([P, 2 * QC], BF16, tag="pt",
                                      name=f"pt_{hp}_{qc}_{j}_{hh}")
                    pts[hh] = pt
                    if not diag:
                        nc.scalar.activation(
                            pt[:], tiles[hh][:], AF.Exp, scale=RSQRT
                        )
                    else:
                        r0 = k0 - 4 * qc
                        off0, off1 = r0 * P, (r0 + 1) * P
                        nc.scalar.activation(
                            pt[:, off0 : 2 * QC],
                            tiles[hh][:, off0 : 2 * QC],
                            AF.Exp,
                            scale=RSQRT,
                        )
                        if off0 > 0:
                            nc.vector.tensor_copy(
                                pt[:, 0:off0], zeros512[:, 0:off0]
                            )
                        nc.vector.tensor_copy(
                            pt[:, QC : QC + off1], zeros512[:, 0:off1]
                        )
                        nc.vector.tensor_mul(
                            pt[:, off0 : off0 + P],
                            pt[:, off0 : off0 + P],
                            mask01[:],
                        )
                        nc.vector.tensor_mul(
                            pt[:, QC + off1 : QC + off1 + P],
                            pt[:, QC + off1 : QC + off1 + P],
                            mask01[:],
                        )
                av_fifo.append(
                    dict(hp=hp, qc=qc, nkt=nkt, pts=pts, kts=(k0, k0 + 1))
                )
                if len(av_fifo) > 2:  # AV trails S by 2 steps (exp latency)
                    emit_av_step()
                for _ in range(budget[hp]):
                    advance_filler()
        if hp < NP - 1:
            finish_filler(("qk", hp + 1))  # next pair's QK must be complete
            pair_cur = pair_nxt
    while av_fifo:  # drain trailing AV steps
        emit_av_step()
    while advance_filler():  # norms + remaining proj
        pass

    ostage_pool.release()
    rcol_pool.release()
    dcol_pool.release()
    pt_pool.release()
    wqk_pool.release()
    qk_pool.release()
    wp_pool.release()
    oT_pool.release()
    vaug_pool.release()
    wv_pool.release()
    xt_pool.release()
    psum_acc.release()
    psum_mm.release()
    ps_s.release()
    const_pool.release()


_NC_CACHE = None


def _get_program():
    global _NC_CACHE
    if _NC_CACHE is None:
        _NC_CACHE = _build_core_program()
    return _NC_CACHE


BF = ml_dtypes.bfloat16


def _make_in_maps(x, w_qkv, w_proj):
    x = np.asarray(x, dtype=np.float32)
    w_qkv = np.asarray(w_qkv, dtype=np.float32)
    w_proj = np.asarray(w_proj, dtype=np.float32)
    in_maps = []
    for core in range(N_CORES):
        b, g = core // 2, core % 2
        cs = slice(g * COLS, (g + 1) * COLS)
        sel33 = np.zeros((HD, P), dtype=np.float32)
        sel33[0, 0:HD] = 1.0
        sel33[32, HD:P] = 1.0
        in_maps.append(
            {
                "xT": np.ascontiguousarray(x[b].T).astype(BF),
                "sel": sel33,
                "wq": np.ascontiguousarray(
                    w_qkv[:, 0 * DM : 1 * DM][:, cs]
                ).astype(BF),
                "wk": np.ascontiguousarray(
                    w_qkv[:, 1 * DM : 2 * DM][:, cs]
                ).astype(BF),
                "wv": np.ascontiguousarray(
                    w_qkv[:, 2 * DM : 3 * DM][:, cs]
                ).astype(BF),
                "wp": np.ascontiguousarray(w_proj[cs, :]).astype(BF),
            }
        )
    return in_maps


def run_on_hw(x, w_qkv, w_proj, trace=False, **kwargs):
    """Run the SPMD program on 8 cores; returns (full_output, BassKernelResults)."""
    nc = _get_program()
    in_maps = _make_in_maps(x, w_qkv, w_proj)
    res = run_bass_kernel_spmd(
        nc, in_maps, list(range(N_CORES)), trace=trace, **kwargs
    )
    bs = 4
    outp = np.empty((bs, SEQ, DM), dtype=np.float32)
    for b in range(bs):
        outp[b] = res.results[2 * b]["out"] + res.results[2 * b + 1]["out"]
    return outp, res


def kernel(x, w_qkv, w_proj):
    outp, _ = run_on_hw(x, w_qkv, w_proj, trace=False)
    return outp


# revision 39
# speedup vs baseline: 1.2848x; 1.0459x over previous
"""Causal multi-head self-attention on 8 trn2 NeuronCores.

Sharding: 8 cores = 4 batch x 2 head-groups. Core i handles batch i//2 and
heads (i%2)*8 .. (i%2)*8+8 (8 of 16 heads, 512 of 1024 d_model columns).
Each core computes a full (2048, 1024) partial output (its head group pushed
through its w_proj row-slice); the host sums the two partials per batch
element (the tensor-parallel all-reduce done host-side).

All inputs are converted to bf16 on the host; x is host-transposed so x^T
DMAs straight into SBUF (no PE transposes, no staging casts). Per-core
dataflow, everything in transposed layout:
  Q^T, K^T   : w_q/w_k stationary, x^T moving  -> [cols, seq] bf16
  V_aug      : x^T stationary, w_v moving      -> natural [seq, cols] bf16
               + a ones-column per head so the softmax denominator rides the
               AV matmul as output row 64
  S^T        : K^T stationary, Q^T moving; the two heads of a pair go to PE
               row groups 0-63 / 64-127 (K=64) and run concurrently; two
               k-tiles of S land in one 2-bank PSUM tile [128, 1024]
  P^T        : one exp per (head, kt-pair) over the 2-bank tile on ScalarE
               (no max-subtraction: |S|*rsqrt < ~10); causal = skip k>q
               chunks, zero invalid prefixes, 0/1 mask mul on diag blocks
  O^T_aug    : V_aug stationary, P^T moving, accumulated over k-tiles in PSUM
  normalize  : denominators for (h0, h1) of a q-chunk drain to two SBUF
               partitions; 1/d via one reciprocal_approx_fast (DVE), then a
               single K=2 matmul against a 0/1 selector broadcasts both
               reciprocal rows over the pair's 128 partitions; one DVE mul
  out        : O^T stationary, w_proj rows moving -> natural [seq, 1024] f32

Schedule: one strictly-ordered PE chain (dep edges pin every matmul). Per
kt-pair step: 4 S matmuls (pairs packed via row groups), filler units
(next pair's QKV chunks, V tail, normalizes, and for the last head pair the
projection s-tiles), then the previous step's 4 AV matmuls. ScalarE chews
exp one step behind the S matmuls; AV trails exp by a step.
"""

import numpy as np
import ml_dtypes

import concourse.bass as bass
import concourse.mybir as mybir
import concourse.tile as tile
from concourse import bacc
from concourse.bass_utils import run_bass_kernel_spmd
from concourse.masks import make_upper_triangular
from concourse.tile_rust import add_dep_helper
from collections import deque

F32 = mybir.dt.float32
BF16 = mybir.dt.bfloat16
AF = mybir.ActivationFunctionType

SEQ = 2048
DM = 1024
COLS = 512          # head-cols per core (8 heads x 64)
HD = 64
P = 128
N_CORES = 8
RSQRT = 0.125       # 1/sqrt(64)

SEQ_T = SEQ // P    # 16 seq tiles
DM_T = DM // P      # 8 d_model tiles
QC = 512            # q-chunk (PSUM free size)
N_QC = SEQ // QC    # 4 q chunks
NP = 4              # head pairs per core


def _build_core_program():
    nc = bacc.Bacc(
        "TRN2", target_bir_lowering=False, debug=False, num_devices=N_CORES
    )
    xT = nc.dram_tensor("xT", [DM, SEQ], BF16, kind="ExternalInput").ap()
    sel = nc.dram_tensor("sel", [HD, P], F32, kind="ExternalInput").ap()
    wq = nc.dram_tensor("wq", [DM, COLS], BF16, kind="ExternalInput").ap()
    wk = nc.dram_tensor("wk", [DM, COLS], BF16, kind="ExternalInput").ap()
    wv = nc.dram_tensor("wv", [DM, COLS], BF16, kind="ExternalInput").ap()
    wp = nc.dram_tensor("wp", [COLS, DM], BF16, kind="ExternalInput").ap()
    out = nc.dram_tensor("out", [SEQ, DM], F32, kind="ExternalOutput").ap()

    with tile.TileContext(nc) as tc:
        _emit(tc, xT, sel, wq, wk, wv, wp, out)
    nc.compile()
    return nc


def _emit(tc, xT, sel, wq, wk, wv, wp, out):
    nc = tc.nc

    # strict PE order: every matmul chains onto the previous one (order-only
    # edge, no semaphore) so the scheduler cannot interpose PE work between
    # an S row-group pair, which would break their concurrent execution
    chain = [None]

    def pin(mm):
        if chain[0] is not None:
            add_dep_helper(mm.ins, chain[0], sync=False, reason="pe-chain")
        chain[0] = mm.ins

    # --- pools ------------------------------------------------------------
    const_pool = tc.alloc_tile_pool(name="const", bufs=1)
    ps_s = tc.alloc_tile_pool(name="ps_s", bufs=2, space="PSUM")      # 4 banks
    psum_mm = tc.alloc_tile_pool(name="psum_mm", bufs=2, space="PSUM")  # 2
    psum_acc = tc.alloc_tile_pool(name="psum_acc", bufs=2, space="PSUM")  # 2

    # --- constants --------------------------------------------------------
    mask01 = const_pool.tile([P, P], BF16, tag="mask01")
    # 1.0 where free-idx (q) >= partition-idx (k), else 0 — causal in S^T
    make_upper_triangular(nc, mask01[:], val=1.0, diag=True)
    cstage = const_pool.tile([P, QC], F32, tag="cstage")
    nc.vector.memset(cstage[:], 0.0)
    zeros512 = const_pool.tile([P, QC], BF16, tag="zeros512")
    nc.vector.tensor_copy(zeros512[:], cstage[:])
    nc.vector.memset(cstage[:, 0:8], 1.0)
    ones8 = const_pool.tile([P, 8], BF16, tag="ones8")
    nc.vector.tensor_copy(ones8[:], cstage[:, 0:8])
    # 0/1 selector for the reciprocal broadcast (host-built): contraction row
    # 0 carries h0's reciprocal to out partitions 0-63, row 32 carries h1's
    # to 64-127. Staged to f32r — plain-fp32 matmuls split into two HI/LO
    # passes at half rate.
    F32R = mybir.dt.float32r
    sel_stage = const_pool.tile([HD, P], F32, tag="sel_stage")
    nc.sync.dma_start(sel_stage[:], sel[:, :])
    sel64 = const_pool.tile([HD, P], F32R, tag="sel64")
    nc.vector.tensor_copy(sel64[:], sel_stage[:])

    # --- persistent SBUF --------------------------------------------------
    xt_pool = tc.alloc_tile_pool(name="xt", bufs=1)
    xt = xt_pool.tile([P, DM_T * SEQ], BF16, tag="xt")  # x^T, d-tile major
    wv_pool = tc.alloc_tile_pool(name="wv", bufs=1)
    wv_sb = wv_pool.tile([P, DM_T * COLS], BF16, tag="wv_sb")
    vaug_pool = tc.alloc_tile_pool(name="vaug", bufs=1)
    vaug = vaug_pool.tile([P, SEQ_T * 8 * (HD + 1)], BF16, tag="vaug")
    vaug_v = vaug[:].rearrange("p (s h e) -> p s h e", s=SEQ_T, h=8)
    oT_pool = tc.alloc_tile_pool(name="oT", bufs=1)
    oT = oT_pool.tile([P, NP * SEQ], BF16, tag="oT")
    wp_pool = tc.alloc_tile_pool(name="wp", bufs=1)
    wp_sb = wp_pool.tile([P, 4 * DM], BF16, tag="wp_sb")

    qk_pool = tc.alloc_tile_pool(name="qk", bufs=2)
    wqk_pool = tc.alloc_tile_pool(name="wqk", bufs=2)
    pt_pool = tc.alloc_tile_pool(name="pt", bufs=6)
    dcol_pool = tc.alloc_tile_pool(name="dcol", bufs=4)
    rcol_pool = tc.alloc_tile_pool(name="rcol", bufs=4)
    ostage_pool = tc.alloc_tile_pool(name="ostage", bufs=3)

    # --- input DMAs (bf16, direct into final layout) ----------------------
    for d in range(DM_T):
        for h in range(2):  # 16 DMAs spread queues -> faster x^T arrival
            nc.sync.dma_start(
                xt[:, d * SEQ + h * (SEQ // 2) : d * SEQ + (h + 1) * (SEQ // 2)],
                xT[d * P : (d + 1) * P, h * (SEQ // 2) : (h + 1) * (SEQ // 2)],
            )
        nc.sync.dma_start(
            wv_sb[:, d * COLS : (d + 1) * COLS], wv[d * P : (d + 1) * P, :]
        )
    for c in range(4):
        nc.sync.dma_start(
            wp_sb[:, c * DM : (c + 1) * DM], wp[c * P : (c + 1) * P, :]
        )

    # warm-up: keep the PE busy while x^T streams in, so HAM reaches 8/8
    # before the V matmuls start (otherwise the whole head phase runs at
    # 1.2 GHz and the DMA-trickle gaps re-throttle it)
    warm_ps = psum_mm.tile([P, QC], F32, tag="mm", name="warm_ps")
    for i in range(180):
        mm = nc.tensor.matmul(
            warm_ps[:, 0:P], mask01[:], zeros512[:, 0:P], start=True, stop=True
        )
        pin(mm)

    # ===== V (natural layout) + ones columns ==============================
    def v_mms(s):
        ps = psum_mm.tile([P, QC], F32, tag="mm", name=f"vps_{s}")
        for d in range(DM_T):
            mm = nc.tensor.matmul(
                ps[:],
                xt[:, d * SEQ + s * P : d * SEQ + (s + 1) * P],
                wv_sb[:, d * COLS : (d + 1) * COLS],
                start=(d == 0),
                stop=(d == DM_T - 1),
            )
            pin(mm)
            if d % 2 == 1 and d < DM_T - 1:
                yield
        nc.vector.tensor_copy(
            vaug_v[:, s, :, 0:HD], ps[:].rearrange("p (h e) -> p h e", h=8)
        )
        nc.vector.tensor_copy(
            vaug_v[:, s, :, HD : HD + 1],
            ones8[:].rearrange("p (a b) -> p a b", b=1),
        )
        yield

    for s in range(12):  # first 12 seq-tiles upfront; rest as fillers
        for _ in v_mms(s):
            pass

    # ===== Q^T / K^T pair machinery =======================================
    def qk_pair_dma(hp):
        wq_sb = wqk_pool.tile([P, DM_T * P], BF16, tag="wq_sb", name=f"wq{hp}")
        wk_sb = wqk_pool.tile([P, DM_T * P], BF16, tag="wk_sb", name=f"wk{hp}")
        nc.sync.dma_start(
            wq_sb[:].rearrange("p (d c) -> p d c", d=DM_T),
            wq[:, hp * P : (hp + 1) * P].rearrange("(d p) c -> p d c", p=P),
        )
        nc.sync.dma_start(
            wk_sb[:].rearrange("p (d c) -> p d c", d=DM_T),
            wk[:, hp * P : (hp + 1) * P].rearrange("(d p) c -> p d c", p=P),
        )
        qT = qk_pool.tile([P, SEQ], BF16, tag="qT", name=f"qT{hp}")
        kT = qk_pool.tile([P, SEQ], BF16, tag="kT", name=f"kT{hp}")
        return dict(wq_sb=wq_sb, wk_sb=wk_sb, qT=qT, kT=kT, hp=hp)

    def qk_chunks(pair):
        # k chunk n before q chunk n; chunk n is needed when attention
        # reaches q-chunk n of this pair
        for n in range(N_QC):
            for wsb, dst in (
                (pair["wk_sb"], pair["kT"]),
                (pair["wq_sb"], pair["qT"]),
            ):
                ps = psum_mm.tile(
                    [P, QC], F32, tag="mm", name=f"qkps_{pair['hp']}_{n}"
                )
                for d in range(DM_T):
                    mm = nc.tensor.matmul(
                        ps[:],
                        wsb[:, d * P : (d + 1) * P],
                        xt[:, d * SEQ + n * QC : d * SEQ + (n + 1) * QC],
                        start=(d == 0),
                        stop=(d == DM_T - 1),
                    )
                    pin(mm)
                    if d % 2 == 1 and d < DM_T - 1:
                        yield
                nc.vector.tensor_copy(dst[:, n * QC : (n + 1) * QC], ps[:])
                yield

    # ===== filler queue ===================================================
    fillers = deque()  # (key, generator) — advanced one unit at a time
    step_no = [0]
    delayed = []  # (ready_step, (key, gen)) — kept in append order

    def mature_delayed():
        while delayed and delayed[0][0] <= step_no[0]:
            fillers.append(delayed.pop(0)[1])

    def advance_filler():
        while fillers:
            key, gen = fillers[0]
            try:
                next(gen)
                return True
            except StopIteration:
                fillers.popleft()
        return False

    def finish_filler(want_key):
        for key, gen in list(fillers):
            if key == want_key:
                for _ in gen:
                    pass
                fillers.remove((key, gen))

    def v_rest_gen():
        for s in range(12, SEQ_T):
            for _ in v_mms(s):
                yield

    def norm_gen(hp, qc, dcol):
        # 1/d on DVE (no ScalarE table thrash): rows 0/32 hold the two
        # heads' denominators; one K=33 matmul against the 0/1 selector
        # broadcasts both reciprocal rows over the pair's 128 partitions.
        # Unit 1 is DVE-only so the pinned PE chain never waits on it.
        rcol = rcol_pool.tile([P, QC], F32, tag="rcol", name=f"rc_{hp}_{qc}")
        rcr = rcol_pool.tile([P, QC], mybir.dt.float32r, tag="rcr",
                             name=f"rcr_{hp}_{qc}")
        nc.vector.reciprocal_approx_fast(rcol[0:HD, :], dcol[0:HD, :])
        nc.vector.tensor_copy(rcr[0:HD, :], rcol[0:HD, :])
        yield
        rc = psum_mm.tile([P, QC], F32, tag="mm", name=f"rcb_{hp}_{qc}")
        mm = nc.tensor.matmul(
            rc[:], sel64[0:HD, :], rcr[0:HD, :], start=True, stop=True
        )
        pin(mm)
        sl = oT[:, hp * SEQ + qc * QC : hp * SEQ + (qc + 1) * QC]
        nc.vector.tensor_mul(sl, sl, rc[:])
        yield

    def proj_gen(s):
        ost = ostage_pool.tile([P, DM], F32, tag="ost", name=f"ost_{s}")
        for n2 in range(2):
            ps = psum_mm.tile([P, QC], F32, tag="mm", name=f"pps_{s}_{n2}")
            for c in range(4):
                mm = nc.tensor.matmul(
                    ps[:],
                    oT[:, c * SEQ + s * P : c * SEQ + (s + 1) * P],
                    wp_sb[:, c * DM + n2 * QC : c * DM + (n2 + 1) * QC],
                    start=(c == 0),
                    stop=(c == 3),
                )
                pin(mm)
                if c == 1:
                    yield
            nc.vector.tensor_copy(ost[:, n2 * QC : (n2 + 1) * QC], ps[:])
            yield
        nc.sync.dma_start(out[s * P : (s + 1) * P, :], ost[:])

    # ===== attention ======================================================
    av_fifo = deque()
    po_cur = {}
    dcol_cur = [None]

    def emit_av_step():
        if not av_fifo:
            return
        rec = av_fifo.popleft()
        hp, qc, nkt, pts = rec["hp"], rec["qc"], rec["nkt"], rec["pts"]
        for idx, kt in enumerate(rec["kts"]):
            for hh in range(2):
                if kt == 0:
                    po_cur[hh] = psum_acc.tile(
                        [P, QC], F32, tag="po", name=f"po_{hp}_{qc}_{hh}"
                    )
                po = po_cur[hh]
                mm = nc.tensor.matmul(
                    po[0 : HD + 1, :],
                    vaug_v[:, kt, 2 * hp + hh, :],
                    pts[hh][:, idx * QC : (idx + 1) * QC],
                    start=(kt == 0),
                    stop=(kt == nkt - 1),
                )
                pin(mm)
                if kt == nkt - 1:
                    if hh == 0:
                        dcol_cur[0] = dcol_pool.tile(
                            [P, QC], F32, tag="dcol", name=f"dc_{hp}_{qc}"
                        )
                        # rows 1-31 must be finite for the [0:33] reciprocal
                        nc.vector.memset(dcol_cur[0][0:HD, :], 1.0)
                    dcol = dcol_cur[0]
                    nc.vector.tensor_copy(
                        dcol[32 * hh : 32 * hh + 1, :], po[HD : HD + 1, :]
                    )
                    nc.vector.tensor_copy(
                        oT[
                            hh * HD : (hh + 1) * HD,
                            hp * SEQ + qc * QC : hp * SEQ + (qc + 1) * QC,
                        ],
                        po[0:HD, :],
                    )
                    if hh == 1:
                        # normalize must not enter the in-order PE queue
                        # until its DVE reciprocal chain has had time to
                        # drain (head-of-line blocking): mature 2 steps out
                        delayed.append(
                            (step_no[0] + 2, ("norm", norm_gen(hp, qc, dcol)))
                        )
                        if hp == NP - 1:
                            for s in range(4 * qc, 4 * qc + 4):
                                delayed.append(
                                    (step_no[0] + 2, ("proj", proj_gen(s)))
                                )

    pair_cur = qk_pair_dma(0)
    for _ in qk_chunks(pair_cur):  # pair 0 fully upfront
        pass

    budget = {0: 4, 1: 3, 2: 3, 3: 3}
    for hp in range(NP):
        if hp == 0:
            fillers.append(("vrest", v_rest_gen()))
        if hp < NP - 1:
            pair_nxt = qk_pair_dma(hp + 1)
            fillers.append((("qk", hp + 1), qk_chunks(pair_nxt)))
        qT, kT = pair_cur["qT"], pair_cur["kT"]
        for qc in range(N_QC):
            nkt = 4 * qc + 4
            for j in range(nkt // 2):
                k0 = 2 * j
                A = ps_s.tile([P, 2 * QC], F32, tag="s2",
                              name=f"sA_{hp}_{qc}_{j}")
                B = ps_s.tile([P, 2 * QC], F32, tag="s2",
                              name=f"sB_{hp}_{qc}_{j}")
                tiles = {0: A, 1: B}
                for idx, kt in enumerate((k0, k0 + 1)):
                    for hh in range(2):
                        b = hh * HD
                        mm = nc.tensor.matmul(
                            tiles[hh][:, idx * QC : (idx + 1) * QC],
                            kT[b : b + HD, kt * P : (kt + 1) * P],
                            qT[b : b + HD, qc * QC : (qc + 1) * QC],
                            start=True,
                            stop=True,
                        )
                        pin(mm)
                pts = {}
                diag = k0 >= 4 * qc
                for hh in range(2):
                    pt = pt_pool.tile([P, 2 * QC], BF16, tag="pt",
                                      name=f"pt_{hp}_{qc}_{j}_{hh}")
                    pts[hh] = pt
                    if not diag:
                        nc.scalar.activation(
                            pt[:], tiles[hh][:], AF.Exp, scale=RSQRT
                        )
                    else:
                        r0 = k0 - 4 * qc
                        off0, off1 = r0 * P, (r0 + 1) * P
                        nc.scalar.activation(
                            pt[:, off0 : 2 * QC],
                            tiles[hh][:, off0 : 2 * QC],
                            AF.Exp,
                            scale=RSQRT,
                        )
                        if off0 > 0:
                            nc.vector.tensor_copy(
                                pt[:, 0:off0], zeros512[:, 0:off0]
                            )
                        nc.vector.tensor_copy(
                            pt[:, QC : QC + off1], zeros512[:, 0:off1]
                        )
                        nc.vector.tensor_mul(
                            pt[:, off0 : off0 + P],
                            pt[:, off0 : off0 + P],
                            mask01[:],
                        )
                        nc.vector.tensor_mul(
                            pt[:, QC + off1 : QC + off1 + P],
                            pt[:, QC + off1 : QC + off1 + P],
                            mask01[:],
                        )
                av_fifo.append(
                    dict(hp=hp, qc=qc, nkt=nkt, pts=pts, kts=(k0, k0 + 1))
                )
                if len(av_fifo) > 2:  # AV trails S by 2 steps (exp latency)
                    emit_av_step()
                step_no[0] += 1
                mature_delayed()
                for _ in range(budget[hp]):
                    advance_filler()
        if hp < NP - 1:
            finish_filler(("qk", hp + 1))  # next pair's QK must be complete
            pair_cur = pair_nxt
    while av_fifo:  # drain trailing AV steps
        emit_av_step()
    step_no[0] += 1000
    mature_delayed()
    while advance_filler():  # norms + remaining proj
        pass

    ostage_pool.release()
    rcol_pool.release()
    dcol_pool.release()
    pt_pool.release()
    wqk_pool.release()
    qk_pool.release()
    wp_pool.release()
    oT_pool.release()
    vaug_pool.release()
    wv_pool.release()
    xt_pool.release()
    psum_acc.release()
    psum_mm.release()
    ps_s.release()
    const_pool.release()


_NC_CACHE = None


def _get_program():
    global _NC_CACHE
    if _NC_CACHE is None:
        _NC_CACHE = _build_core_program()
    return _NC_CACHE


BF = ml_dtypes.bfloat16


def _make_in_maps(x, w_qkv, w_proj):
    x = np.asarray(x, dtype=np.float32)
    w_qkv = np.asarray(w_qkv, dtype=np.float32)
    w_proj = np.asarray(w_proj, dtype=np.float32)
    in_maps = []
    for core in range(N_CORES):
        b, g = core // 2, core % 2
        cs = slice(g * COLS, (g + 1) * COLS)
        sel33 = np.zeros((HD, P), dtype=np.float32)
        sel33[0, 0:HD] = 1.0
        sel33[32, HD:P] = 1.0
        in_maps.append(
            {
                "xT": np.ascontiguousarray(x[b].T).astype(BF),
                "sel": sel33,
                "wq": np.ascontiguousarray(
                    w_qkv[:, 0 * DM : 1 * DM][:, cs]
                ).astype(BF),
                "wk": np.ascontiguousarray(
                    w_qkv[:, 1 * DM : 2 * DM][:, cs]
                ).astype(BF),
                "wv": np.ascontiguousarray(
                    w_qkv[:, 2 * DM : 3 * DM][:, cs]
                ).astype(BF),
                "wp": np.ascontiguousarray(w_proj[cs, :]).astype(BF),
            }
        )
    return in_maps


def run_on_hw(x, w_qkv, w_proj, trace=False, **kwargs):
    """Run the SPMD program on 8 cores; returns (full_output, BassKernelResults)."""
    nc = _get_program()
    in_maps = _make_in_maps(x, w_qkv, w_proj)
    res = run_bass_kernel_spmd(
        nc, in_maps, list(range(N_CORES)), trace=trace, **kwargs
    )
    bs = 4
    outp = np.empty((bs, SEQ, DM), dtype=np.float32)
    for b in range(bs):
        outp[b] = res.results[2 * b]["out"] + res.results[2 * b + 1]["out"]
    return outp, res


def kernel(x, w_qkv, w_proj):
    outp, _ = run_on_hw(x, w_qkv, w_proj, trace=False)
    return outp


# revision 42
# speedup vs baseline: 1.2848x; 1.0000x over previous
"""Causal multi-head self-attention on 8 trn2 NeuronCores.

Sharding: 8 cores = 4 batch x 2 head-groups. Core i handles batch i//2 and
heads (i%2)*8 .. (i%2)*8+8 (8 of 16 heads, 512 of 1024 d_model columns).
Each core computes a full (2048, 1024) partial output (its head group pushed
through its w_proj row-slice); the host sums the two partials per batch
element (the tensor-parallel all-reduce done host-side).

All inputs are converted to bf16 on the host; x is host-transposed so x^T
DMAs straight into SBUF (no PE transposes, no staging casts). Per-core
dataflow, everything in transposed layout:
  Q^T, K^T   : w_q/w_k stationary, x^T moving  -> [cols, seq] bf16
  V_aug      : x^T stationary, w_v moving      -> natural [seq, cols] bf16
               + a ones-column per head so the softmax denominator rides the
               AV matmul as output row 64
  S^T        : K^T stationary, Q^T moving; the two heads of a pair go to PE
               row groups 0-63 / 64-127 (K=64) and run concurrently; two
               k-tiles of S land in one 2-bank PSUM tile [128, 1024]
  P^T        : one exp per (head, kt-pair) over the 2-bank tile on ScalarE
               (no max-subtraction: |S|*rsqrt < ~10); causal = skip k>q
               chunks, zero invalid prefixes, 0/1 mask mul on diag blocks
  O^T_aug    : V_aug stationary, P^T moving, accumulated over k-tiles in PSUM
  normalize  : denominators for (h0, h1) of a q-chunk drain to two SBUF
               partitions; 1/d via one reciprocal_approx_fast (DVE), then a
               single K=2 matmul against a 0/1 selector broadcasts both
               reciprocal rows over the pair's 128 partitions; one DVE mul
  out        : O^T stationary, w_proj rows moving -> natural [seq, 1024] f32

Schedule: one strictly-ordered PE chain (dep edges pin every matmul). Per
kt-pair step: 4 S matmuls (pairs packed via row groups), filler units
(next pair's QKV chunks, V tail, normalizes, and for the last head pair the
projection s-tiles), then the previous step's 4 AV matmuls. ScalarE chews
exp one step behind the S matmuls; AV trails exp by a step.
"""

import numpy as np
import ml_dtypes

import concourse.bass as bass
import concourse.mybir as mybir
import concourse.tile as tile
from concourse import bacc
from concourse.bass_utils import run_bass_kernel_spmd
from concourse.masks import make_upper_triangular
from concourse.tile_rust import add_dep_helper
from collections import deque

F32 = mybir.dt.float32
BF16 = mybir.dt.bfloat16
AF = mybir.ActivationFunctionType

SEQ = 2048
DM = 1024
COLS = 512          # head-cols per core (8 heads x 64)
HD = 64
P = 128
N_CORES = 8
RSQRT = 0.125       # 1/sqrt(64)

SEQ_T = SEQ // P    # 16 seq tiles
DM_T = DM // P      # 8 d_model tiles
QC = 512            # q-chunk (PSUM free size)
N_QC = SEQ // QC    # 4 q chunks
NP = 4              # head pairs per core


def _build_core_program():
    nc = bacc.Bacc(
        "TRN2", target_bir_lowering=False, debug=False, num_devices=N_CORES
    )
    xT = nc.dram_tensor("xT", [DM, SEQ], BF16, kind="ExternalInput").ap()
    sel = nc.dram_tensor("sel", [HD, P], F32, kind="ExternalInput").ap()
    wq = nc.dram_tensor("wq", [DM, COLS], BF16, kind="ExternalInput").ap()
    wk = nc.dram_tensor("wk", [DM, COLS], BF16, kind="ExternalInput").ap()
    wv = nc.dram_tensor("wv", [DM, COLS], BF16, kind="ExternalInput").ap()
    wp = nc.dram_tensor("wp", [COLS, DM], BF16, kind="ExternalInput").ap()
    out = nc.dram_tensor("out", [SEQ, DM], F32, kind="ExternalOutput").ap()

    with tile.TileContext(nc) as tc:
        _emit(tc, xT, sel, wq, wk, wv, wp, out)
    nc.compile()
    return nc


def _emit(tc, xT, sel, wq, wk, wv, wp, out):
    nc = tc.nc

    # strict PE order: every matmul chains onto the previous one (order-only
    # edge, no semaphore) so the scheduler cannot interpose PE work between
    # an S row-group pair, which would break their concurrent execution
    chain = [None]

    def pin(mm):
        if chain[0] is not None:
            add_dep_helper(mm.ins, chain[0], sync=False, reason="pe-chain")
        chain[0] = mm.ins

    # --- pools ------------------------------------------------------------
    const_pool = tc.alloc_tile_pool(name="const", bufs=1)
    ps_s = tc.alloc_tile_pool(name="ps_s", bufs=2, space="PSUM")      # 4 banks
    psum_mm = tc.alloc_tile_pool(name="psum_mm", bufs=2, space="PSUM")  # 2
    psum_acc = tc.alloc_tile_pool(name="psum_acc", bufs=2, space="PSUM")  # 2

    # --- constants --------------------------------------------------------
    mask01 = const_pool.tile([P, P], BF16, tag="mask01")
    # 1.0 where free-idx (q) >= partition-idx (k), else 0 — causal in S^T
    make_upper_triangular(nc, mask01[:], val=1.0, diag=True)
    cstage = const_pool.tile([P, QC], F32, tag="cstage")
    nc.vector.memset(cstage[:], 0.0)
    zeros512 = const_pool.tile([P, QC], BF16, tag="zeros512")
    nc.vector.tensor_copy(zeros512[:], cstage[:])
    nc.vector.memset(cstage[:, 0:8], 1.0)
    ones8 = const_pool.tile([P, 8], BF16, tag="ones8")
    nc.vector.tensor_copy(ones8[:], cstage[:, 0:8])
    # 0/1 selector for the reciprocal broadcast (host-built): contraction row
    # 0 carries h0's reciprocal to out partitions 0-63, row 32 carries h1's
    # to 64-127. Staged to f32r — plain-fp32 matmuls split into two HI/LO
    # passes at half rate.
    F32R = mybir.dt.float32r
    sel_stage = const_pool.tile([HD, P], F32, tag="sel_stage")
    nc.sync.dma_start(sel_stage[:], sel[:, :])
    sel64 = const_pool.tile([HD, P], F32R, tag="sel64")
    nc.vector.tensor_copy(sel64[:], sel_stage[:])

    # --- persistent SBUF --------------------------------------------------
    xt_pool = tc.alloc_tile_pool(name="xt", bufs=1)
    xt = xt_pool.tile([P, DM_T * SEQ], BF16, tag="xt")  # x^T, d-tile major
    wv_pool = tc.alloc_tile_pool(name="wv", bufs=1)
    wv_sb = wv_pool.tile([P, DM_T * COLS], BF16, tag="wv_sb")
    vaug_pool = tc.alloc_tile_pool(name="vaug", bufs=1)
    vaug = vaug_pool.tile([P, SEQ_T * 8 * (HD + 1)], BF16, tag="vaug")
    vaug_v = vaug[:].rearrange("p (s h e) -> p s h e", s=SEQ_T, h=8)
    oT_pool = tc.alloc_tile_pool(name="oT", bufs=1)
    oT = oT_pool.tile([P, NP * SEQ], BF16, tag="oT")
    wp_pool = tc.alloc_tile_pool(name="wp", bufs=1)
    wp_sb = wp_pool.tile([P, 4 * DM], BF16, tag="wp_sb")

    qk_pool = tc.alloc_tile_pool(name="qk", bufs=2)
    wqk_pool = tc.alloc_tile_pool(name="wqk", bufs=2)
    pt_pool = tc.alloc_tile_pool(name="pt", bufs=6)
    dcol_pool = tc.alloc_tile_pool(name="dcol", bufs=4)
    rcol_pool = tc.alloc_tile_pool(name="rcol", bufs=4)
    ostage_pool = tc.alloc_tile_pool(name="ostage", bufs=3)

    # --- input DMAs (bf16, direct into final layout) ----------------------
    for d in range(DM_T):
        for h in range(2):  # 16 DMAs spread queues -> faster x^T arrival
            nc.sync.dma_start(
                xt[:, d * SEQ + h * (SEQ // 2) : d * SEQ + (h + 1) * (SEQ // 2)],
                xT[d * P : (d + 1) * P, h * (SEQ // 2) : (h + 1) * (SEQ // 2)],
            )
        nc.sync.dma_start(
            wv_sb[:, d * COLS : (d + 1) * COLS], wv[d * P : (d + 1) * P, :]
        )
    for c in range(4):
        nc.sync.dma_start(
            wp_sb[:, c * DM : (c + 1) * DM], wp[c * P : (c + 1) * P, :]
        )

    # warm-up: keep the PE busy while x^T streams in, so HAM reaches 8/8
    # before the V matmuls start (otherwise the whole head phase runs at
    # 1.2 GHz and the DMA-trickle gaps re-throttle it)
    warm_ps = psum_mm.tile([P, QC], F32, tag="mm", name="warm_ps")
    for i in range(180):
        mm = nc.tensor.matmul(
            warm_ps[:, 0:P], mask01[:], zeros512[:, 0:P], start=True, stop=True
        )
        pin(mm)

    # ===== V (natural layout) + ones columns ==============================
    def v_mms(s):
        ps = psum_mm.tile([P, QC], F32, tag="mm", name=f"vps_{s}")
        for d in range(DM_T):
            mm = nc.tensor.matmul(
                ps[:],
                xt[:, d * SEQ + s * P : d * SEQ + (s + 1) * P],
                wv_sb[:, d * COLS : (d + 1) * COLS],
                start=(d == 0),
                stop=(d == DM_T - 1),
            )
            pin(mm)
            if d % 2 == 1 and d < DM_T - 1:
                yield
        nc.vector.tensor_copy(
            vaug_v[:, s, :, 0:HD], ps[:].rearrange("p (h e) -> p h e", h=8)
        )
        nc.vector.tensor_copy(
            vaug_v[:, s, :, HD : HD + 1],
            ones8[:].rearrange("p (a b) -> p a b", b=1),
        )
        yield

    for s in range(12):  # first 12 seq-tiles upfront; rest as fillers
        for _ in v_mms(s):
            pass

    # ===== Q^T / K^T pair machinery =======================================
    def qk_pair_dma(hp):
        wq_sb = wqk_pool.tile([P, DM_T * P], BF16, tag="wq_sb", name=f"wq{hp}")
        wk_sb = wqk_pool.tile([P, DM_T * P], BF16, tag="wk_sb", name=f"wk{hp}")
        nc.sync.dma_start(
            wq_sb[:].rearrange("p (d c) -> p d c", d=DM_T),
            wq[:, hp * P : (hp + 1) * P].rearrange("(d p) c -> p d c", p=P),
        )
        nc.sync.dma_start(
            wk_sb[:].rearrange("p (d c) -> p d c", d=DM_T),
            wk[:, hp * P : (hp + 1) * P].rearrange("(d p) c -> p d c", p=P),
        )
        qT = qk_pool.tile([P, SEQ], BF16, tag="qT", name=f"qT{hp}")
        kT = qk_pool.tile([P, SEQ], BF16, tag="kT", name=f"kT{hp}")
        return dict(wq_sb=wq_sb, wk_sb=wk_sb, qT=qT, kT=kT, hp=hp)

    def qk_chunks(pair):
        # k chunk n before q chunk n; chunk n is needed when attention
        # reaches q-chunk n of this pair
        for n in range(N_QC):
            for wsb, dst in (
                (pair["wk_sb"], pair["kT"]),
                (pair["wq_sb"], pair["qT"]),
            ):
                ps = psum_mm.tile(
                    [P, QC], F32, tag="mm", name=f"qkps_{pair['hp']}_{n}"
                )
                for d in range(DM_T):
                    mm = nc.tensor.matmul(
                        ps[:],
                        wsb[:, d * P : (d + 1) * P],
                        xt[:, d * SEQ + n * QC : d * SEQ + (n + 1) * QC],
                        start=(d == 0),
                        stop=(d == DM_T - 1),
                    )
                    pin(mm)
                    if d % 2 == 1 and d < DM_T - 1:
                        yield
                nc.vector.tensor_copy(dst[:, n * QC : (n + 1) * QC], ps[:])
                yield

    # ===== filler queue ===================================================
    fillers = deque()  # (key, generator) — advanced one unit at a time
    step_no = [0]
    delayed = []  # (ready_step, (key, gen)) — kept in append order

    def mature_delayed():
        while delayed and delayed[0][0] <= step_no[0]:
            fillers.append(delayed.pop(0)[1])

    def advance_filler():
        while fillers:
            key, gen = fillers[0]
            try:
                next(gen)
                return True
            except StopIteration:
                fillers.popleft()
        return False

    def finish_filler(want_key):
        for key, gen in list(fillers):
            if key == want_key:
                for _ in gen:
                    pass
                fillers.remove((key, gen))

    def v_rest_gen():
        for s in range(12, SEQ_T):
            for _ in v_mms(s):
                yield

    def norm_gen(hp, qc, dcol):
        # 1/d on DVE (no ScalarE table thrash): rows 0/32 hold the two
        # heads' denominators; one K=33 matmul against the 0/1 selector
        # broadcasts both reciprocal rows over the pair's 128 partitions.
        # Unit 1 is DVE-only so the pinned PE chain never waits on it.
        rcol = rcol_pool.tile([P, QC], F32, tag="rcol", name=f"rc_{hp}_{qc}")
        rcr = rcol_pool.tile([P, QC], mybir.dt.float32r, tag="rcr",
                             name=f"rcr_{hp}_{qc}")
        nc.vector.reciprocal_approx_fast(rcol[0:HD, :], dcol[0:HD, :])
        nc.vector.tensor_copy(rcr[0:HD, :], rcol[0:HD, :])
        yield
        rc = psum_mm.tile([P, QC], F32, tag="mm", name=f"rcb_{hp}_{qc}")
        mm = nc.tensor.matmul(
            rc[:], sel64[0:HD, :], rcr[0:HD, :], start=True, stop=True
        )
        pin(mm)
        sl = oT[:, hp * SEQ + qc * QC : hp * SEQ + (qc + 1) * QC]
        nc.vector.tensor_mul(sl, sl, rc[:])
        yield

    def proj_gen(s):
        ost = ostage_pool.tile([P, DM], F32, tag="ost", name=f"ost_{s}")
        for n2 in range(2):
            ps = psum_mm.tile([P, QC], F32, tag="mm", name=f"pps_{s}_{n2}")
            for c in range(4):
                mm = nc.tensor.matmul(
                    ps[:],
                    oT[:, c * SEQ + s * P : c * SEQ + (s + 1) * P],
                    wp_sb[:, c * DM + n2 * QC : c * DM + (n2 + 1) * QC],
                    start=(c == 0),
                    stop=(c == 3),
                )
                pin(mm)
                if c == 1:
                    yield
            nc.vector.tensor_copy(ost[:, n2 * QC : (n2 + 1) * QC], ps[:])
            yield
        nc.sync.dma_start(out[s * P : (s + 1) * P, :], ost[:])

    # ===== attention ======================================================
    av_fifo = deque()
    po_cur = {}
    dcol_cur = [None]

    def emit_av_step():
        if not av_fifo:
            return
        rec = av_fifo.popleft()
        hp, qc, nkt, pts = rec["hp"], rec["qc"], rec["nkt"], rec["pts"]
        for idx, kt in enumerate(rec["kts"]):
            for hh in range(2):
                if kt == 0:
                    po_cur[hh] = psum_acc.tile(
                        [P, QC], F32, tag="po", name=f"po_{hp}_{qc}_{hh}"
                    )
                po = po_cur[hh]
                mm = nc.tensor.matmul(
                    po[0 : HD + 1, :],
                    vaug_v[:, kt, 2 * hp + hh, :],
                    pts[hh][:, idx * QC : (idx + 1) * QC],
                    start=(kt == 0),
                    stop=(kt == nkt - 1),
                )
                pin(mm)
                if kt == nkt - 1:
                    if hh == 0:
                        dcol_cur[0] = dcol_pool.tile(
                            [P, QC], F32, tag="dcol", name=f"dc_{hp}_{qc}"
                        )
                        # rows 1-31 must be finite for the [0:33] reciprocal
                        nc.vector.memset(dcol_cur[0][0:HD, :], 1.0)
                    dcol = dcol_cur[0]
                    nc.vector.tensor_copy(
                        dcol[32 * hh : 32 * hh + 1, :], po[HD : HD + 1, :]
                    )
                    nc.vector.tensor_copy(
                        oT[
                            hh * HD : (hh + 1) * HD,
                            hp * SEQ + qc * QC : hp * SEQ + (qc + 1) * QC,
                        ],
                        po[0:HD, :],
                    )
                    if hh == 1:
                        # normalize must not enter the in-order PE queue
                        # until its DVE reciprocal chain has had time to
                        # drain (head-of-line blocking): mature 2 steps out
                        delayed.append(
                            (step_no[0] + 2, ("norm", norm_gen(hp, qc, dcol)))
                        )
                        if hp == NP - 1:
                            for s in range(4 * qc, 4 * qc + 4):
                                delayed.append(
                                    (step_no[0] + 2, ("proj", proj_gen(s)))
                                )

    pair_cur = qk_pair_dma(0)
    for _ in qk_chunks(pair_cur):  # pair 0 fully upfront
        pass

    budget = {0: 4, 1: 3, 2: 3, 3: 3}
    for hp in range(NP):
        if hp == 0:
            fillers.append(("vrest", v_rest_gen()))
        if hp < NP - 1:
            pair_nxt = qk_pair_dma(hp + 1)
            fillers.append((("qk", hp + 1), qk_chunks(pair_nxt)))
        qT, kT = pair_cur["qT"], pair_cur["kT"]
        for qc in range(N_QC):
            nkt = 4 * qc + 4
            for j in range(nkt // 2):
                k0 = 2 * j
                # AV first: its pt inputs are 2 steps old (always ready), and
                # its ~1us of PE work covers the previous step's h1-exp tail
                # so the S tiles' PSUM slots are free when S4 issues
                if len(av_fifo) >= 2:
                    emit_av_step()
                A = ps_s.tile([P, 2 * QC], F32, tag="s2",
                              name=f"sA_{hp}_{qc}_{j}")
                B = ps_s.tile([P, 2 * QC], F32, tag="s2",
                              name=f"sB_{hp}_{qc}_{j}")
                tiles = {0: A, 1: B}
                for idx, kt in enumerate((k0, k0 + 1)):
                    for hh in range(2):
                        b = hh * HD
                        mm = nc.tensor.matmul(
                            tiles[hh][:, idx * QC : (idx + 1) * QC],
                            kT[b : b + HD, kt * P : (kt + 1) * P],
                            qT[b : b + HD, qc * QC : (qc + 1) * QC],
                            start=True,
                            stop=True,
                        )
                        pin(mm)
                pts = {}
                diag = k0 >= 4 * qc
                for hh in range(2):
                    pt = pt_pool.tile([P, 2 * QC], BF16, tag="pt",
                                      name=f"pt_{hp}_{qc}_{j}_{hh}")
                    pts[hh] = pt
                    if not diag:
                        nc.scalar.activation(
                            pt[:], tiles[hh][:], AF.Exp, scale=RSQRT
                        )
                    else:
                        r0 = k0 - 4 * qc
                        off0, off1 = r0 * P, (r0 + 1) * P
                        nc.scalar.activation(
                            pt[:, off0 : 2 * QC],
                            tiles[hh][:, off0 : 2 * QC],
                            AF.Exp,
                            scale=RSQRT,
                        )
                        if off0 > 0:
                            nc.vector.tensor_copy(
                                pt[:, 0:off0], zeros512[:, 0:off0]
                            )
                        nc.vector.tensor_copy(
                            pt[:, QC : QC + off1], zeros512[:, 0:off1]
                        )
                        nc.vector.tensor_mul(
                            pt[:, off0 : off0 + P],
                            pt[:, off0 : off0 + P],
                            mask01[:],
                        )
                        nc.vector.tensor_mul(
                            pt[:, QC + off1 : QC + off1 + P],
                            pt[:, QC + off1 : QC + off1 + P],
                            mask01[:],
                        )
                av_fifo.append(
                    dict(hp=hp, qc=qc, nkt=nkt, pts=pts, kts=(k0, k0 + 1))
                )
                step_no[0] += 1
                mature_delayed()
                for _ in range(budget[hp]):
                    advance_filler()
        if hp < NP - 1:
            finish_filler(("qk", hp + 1))  # next pair's QK must be complete
            pair_cur = pair_nxt
    while av_fifo:  # drain trailing AV steps, interleaving filler work
        emit_av_step()
        step_no[0] += 1
        mature_delayed()
        advance_filler()
    step_no[0] += 1000
    mature_delayed()
    while advance_filler():  # norms + remaining proj
        pass

    ostage_pool.release()
    rcol_pool.release()
    dcol_pool.release()
    pt_pool.release()
    wqk_pool.release()
    qk_pool.release()
    wp_pool.release()
    oT_pool.release()
    vaug_pool.release()
    wv_pool.release()
    xt_pool.release()
    psum_acc.release()
    psum_mm.release()
    ps_s.release()
    const_pool.release()


_NC_CACHE = None


def _get_program():
    global _NC_CACHE
    if _NC_CACHE is None:
        _NC_CACHE = _build_core_program()
    return _NC_CACHE


BF = ml_dtypes.bfloat16


def _make_in_maps(x, w_qkv, w_proj):
    x = np.asarray(x, dtype=np.float32)
    w_qkv = np.asarray(w_qkv, dtype=np.float32)
    w_proj = np.asarray(w_proj, dtype=np.float32)
    in_maps = []
    for core in range(N_CORES):
        b, g = core // 2, core % 2
        cs = slice(g * COLS, (g + 1) * COLS)
        sel33 = np.zeros((HD, P), dtype=np.float32)
        sel33[0, 0:HD] = 1.0
        sel33[32, HD:P] = 1.0
        in_maps.append(
            {
                "xT": np.ascontiguousarray(x[b].T).astype(BF),
                "sel": sel33,
                "wq": np.ascontiguousarray(
                    w_qkv[:, 0 * DM : 1 * DM][:, cs]
                ).astype(BF),
                "wk": np.ascontiguousarray(
                    w_qkv[:, 1 * DM : 2 * DM][:, cs]
                ).astype(BF),
                "wv": np.ascontiguousarray(
                    w_qkv[:, 2 * DM : 3 * DM][:, cs]
                ).astype(BF),
                "wp": np.ascontiguousarray(w_proj[cs, :]).astype(BF),
            }
        )
    return in_maps


def run_on_hw(x, w_qkv, w_proj, trace=False, **kwargs):
    """Run the SPMD program on 8 cores; returns (full_output, BassKernelResults)."""
    nc = _get_program()
    in_maps = _make_in_maps(x, w_qkv, w_proj)
    res = run_bass_kernel_spmd(
        nc, in_maps, list(range(N_CORES)), trace=trace, **kwargs
    )
    bs = 4
    outp = np.empty((bs, SEQ, DM), dtype=np.float32)
    for b in range(bs):
        outp[b] = res.results[2 * b]["out"] + res.results[2 * b + 1]["out"]
    return outp, res


def kernel(x, w_qkv, w_proj):
    outp, _ = run_on_hw(x, w_qkv, w_proj, trace=False)
    return outp
